# revision 13
# baseline (speedup 1.0000x reference)
"""BCGNN (nn_BCGNN_15934328668763) Trainium2 Bass kernel, 8 NeuronCores.

Destination-sharded edge-parallel design: core k owns node slice
[k*NS, (k+1)*NS). Every segment-sum lands entirely in the owner's slice, so
no all-reduce of [N,H] partials is needed -- only one AllGather of the
updated h between the two refinement iterations.

Sparsity exploited (vs the dense reference):
  - e_work only changes on tie edges (~5% of E): refine MLP runs on those.
  - m_node_all only takes edges with w_norm != 0 (internal & is_cpl[col]).
  - all [E]/[N] scalar index prep (w_norm, deg, is_cpl, sden) is host-side
    sharding logic; all [*,H] tensor compute runs on device in bf16.
"""
import sys, os

for _p in ('/opt/trn_rl_repo', '/root/.axon_site/_ro/trn_rl_repo'):
    if os.path.isdir(_p) and _p not in sys.path:
        sys.path.insert(0, _p)

import numpy as np
import ml_dtypes

nbf16 = ml_dtypes.bfloat16

# problem constants (hardcoded; harness calls kernel() with exactly these shapes)
N_FULL = 50000
E_FULL = 800000
H = 128
M_CORES = 8
N_ITER = 2
BOUND = 32768          # int16 gather index limit (exclusive)

CA_DEF = 4096          # A-stream (m_node) chunk: indices per dma_gather/scatter
CT_DEF = 512           # tie-stream chunk


# ----------------------------------------------------------------------------
# host-side planning
# ----------------------------------------------------------------------------

def _wrap_idx(mat16):
    """[16, L] channel-wrapped int16 -> [128, L] replicated for 8 Q7 cores."""
    return np.tile(mat16, (8, 1)).astype(np.int16)


def _plan(h, e, edge_attr_raw, edge_index, is_tie, M, CA, CT):
    N, Hd = h.shape
    E = e.shape[0]
    assert Hd == H
    NS = N // M
    SA = CA // 16           # slots per channel per A-chunk
    ST = CT // 16
    row = edge_index[0].astype(np.int64)
    col = edge_index[1].astype(np.int64)
    tie = np.asarray(is_tie).astype(bool)
    tie_f = tie.astype(np.float32)

    cpl_cnt = (np.bincount(row, weights=tie_f, minlength=N)
               + np.bincount(col, weights=tie_f, minlength=N)).astype(np.float32)
    is_cpl = cpl_cnt > 0
    internal = ~tie
    to_cpl = internal & is_cpl[col]
    X = np.abs(edge_attr_raw[:, 1].astype(np.float32))
    w = np.where(to_cpl, (1.0 / np.sqrt(X * X + np.float32(1e-6))).astype(np.float32),
                 np.float32(0.0)).astype(np.float32)
    w_den = np.bincount(col, weights=w, minlength=N).astype(np.float32)
    w_norm = (w / (w_den[col] + np.float32(1e-6))).astype(np.float32)
    deg = np.maximum(cpl_cnt, np.float32(1.0))
    deg_inv = (np.float32(1.0) / deg).astype(np.float32)
    sden = np.bincount(col, weights=w_norm, minlength=N).astype(np.float32)

    core_of = lambda x: x // NS

    # ---------------- A stream (m_node): one-hot matmul accumulation ----------
    # Edges sorted by (row-bucket, dest). Each (bucket, dest-tile) run is padded
    # to a multiple of 128; a 128-position group feeds one one-hot matmul into
    # the dest-tile's PSUM accumulator. Group structure unified across cores.
    NTILE = -(-NS // 128)
    a_sel_all = to_cpl
    a_core = core_of(col)
    NCG = CA // 128
    per_core = []
    cnt = np.zeros((M, 2, NTILE), np.int64)
    for k in range(M):
        sel = a_sel_all & (a_core == k)
        r = row[sel]
        dloc = (col[sel] - k * NS).astype(np.int64)
        wv = w_norm[sel]
        bk = (r >= BOUND).astype(np.int64)
        order = np.lexsort((dloc, bk))
        r, dloc, wv, bk = r[order], dloc[order], wv[order], bk[order]
        dt = dloc // 128
        for b in (0, 1):
            cnt[k, b] = np.bincount(dt[bk == b], minlength=NTILE)
        per_core.append((r, dloc, wv, bk, dt))
    G = np.maximum(-(-cnt.max(axis=0) // 128), 0)        # [2, NTILE] groups per run
    # pad each bucket's group count to a chunk multiple (merge pad into last run)
    for b in (0, 1):
        ng = int(G[b].sum())
        if ng == 0:
            continue
        padg = (-ng) % NCG
        last = np.nonzero(G[b])[0][-1]
        G[b, last] += padg
    NA = int(G.sum()) // NCG
    a_bases = []
    for b in (0, 1):
        a_bases += [b * BOUND] * (int(G[b].sum()) // NCG)
    # group metadata: (dt, start, stop, first_pass) per global group
    a_groups = []
    gfirst = {}          # (b, dt) -> first group index
    gi = 0
    for b in (0, 1):
        for dtv in range(NTILE):
            n = int(G[b, dtv])
            if n == 0:
                continue
            gfirst[(b, dtv)] = gi
            firstpass = (b == 0) or (G[0, dtv] == 0)
            for u in range(n):
                a_groups.append((dtv, u == 0, u == n - 1, firstpass))
                gi += 1
    NGT = gi
    assert NGT == NA * NCG

    A_g, A_w, A_d = [], [], []
    for k in range(M):
        g16 = np.zeros((16, NA * (CA // 16)), np.int16)
        wmat = np.zeros((128, NGT), np.float32)
        dmat = np.zeros((128, NGT), np.float32)
        r, dloc, wv, bk, dt = per_core[k]
        run_id = bk * NTILE + dt
        # start position of each run in the unified stream
        run_base = np.zeros(2 * NTILE, np.int64)
        for b in (0, 1):
            for dtv in range(NTILE):
                if (b, dtv) in gfirst:
                    run_base[b * NTILE + dtv] = 128 * gfirst[(b, dtv)]
        # rank within run
        uniq, inv_start = np.unique(run_id, return_index=True)
        starts = np.zeros(2 * NTILE, np.int64)
        starts[uniq] = inv_start
        rank = np.arange(len(run_id)) - starts[run_id]
        pos = run_base[run_id] + rank
        g16[pos % 16, pos // 16] = (r - bk * BOUND).astype(np.int16)
        wmat[pos % 128, pos // 128] = wv
        dmat[pos % 128, pos // 128] = (dloc - dt * 128).astype(np.float32)
        A_g.append(_wrap_idx(g16))
        A_w.append(wmat)
        A_d.append(dmat)

    # ---------------- tie streams (CL: dest=col, RL: dest=row) ----------------
    tidx_all = np.nonzero(tie)[0]

    def build_tie(dest_is_col):
        # duplicate-free packing: within any chunk each scatter dest appears at
        # most once (HW dma_scatter_add loses updates on same-call duplicates);
        # cross-chunk accumulation is safe (Tile serializes WAW on the buffer).
        per_core = []
        for k in range(M):
            dsrc = col[tidx_all] if dest_is_col else row[tidx_all]
            sel = core_of(dsrc) == k
            te = tidx_all[sel]
            rsrc = row[te]
            csrc = col[te]
            dloc = (dsrc[sel] - k * NS).astype(np.int64)
            grp = (rsrc >= BOUND) * 2 + (csrc >= BOUND)
            entry = []
            for g in range(4):
                gs = grp == g
                entry.append((rsrc[gs], csrc[gs], dloc[gs], te[gs]))
            per_core.append(entry)
        nb = [0, 0, 0, 0]
        for g in range(4):
            NB = 0
            for k in range(M):
                dd = per_core[k][g][2]
                if len(dd) == 0:
                    continue
                mult = int(np.bincount(dd).max())
                NB = max(NB, -(-len(dd) // CT), mult, 1)
            while NB > 0:
                ok = True
                for k in range(M):
                    dd = per_core[k][g][2]
                    if len(dd) == 0:
                        continue
                    order = np.argsort(dd, kind='stable')
                    dds = dd[order]
                    chg = np.r_[0, np.nonzero(np.diff(dds))[0] + 1]
                    rank = np.arange(len(dds)) - np.repeat(chg, np.diff(np.r_[chg, len(dds)]))
                    ch = (rank + dds) % NB
                    if np.bincount(ch, minlength=NB).max() > CT:
                        ok = False
                        break
                if ok:
                    break
                NB += 1
            nb[g] = NB
        NBt = sum(nb)
        gstart = np.concatenate([[0], np.cumsum(nb)[:-1]]).astype(int)
        bases = []
        for g in range(4):
            bases += [((g >> 1) * BOUND, (g & 1) * BOUND)] * nb[g]
        Lr, Lc, Ls, Edat, Eids = [], [], [], [], []
        for k in range(M):
            r16 = np.zeros((16, NBt * ST), np.int16)
            c16 = np.zeros((16, NBt * ST), np.int16)
            s16 = np.full((16, NBt * ST), NS, np.int16)
            ed = np.zeros((128, NBt * CT), nbf16)
            eid = np.full((NBt * CT,), -1, np.int64)
            for g in range(4):
                rr, cc2, dd, ee = per_core[k][g]
                if len(dd) == 0:
                    continue
                order = np.argsort(dd, kind='stable')
                rr, cc2, dd, ee = rr[order], cc2[order], dd[order], ee[order]
                chg = np.r_[0, np.nonzero(np.diff(dd))[0] + 1]
                rank = np.arange(len(dd)) - np.repeat(chg, np.diff(np.r_[chg, len(dd)]))
                ch = (rank + dd) % nb[g]
                o2 = np.argsort(ch, kind='stable')
                rr, cc2, dd, ee, ch = rr[o2], cc2[o2], dd[o2], ee[o2], ch[o2]
                chg2 = np.r_[0, np.nonzero(np.diff(ch))[0] + 1]
                pos = np.arange(len(ch)) - np.repeat(chg2, np.diff(np.r_[chg2, len(ch)]))
                cj = gstart[g] + ch
                r16[pos % 16, cj * ST + pos // 16] = (rr - (g >> 1) * BOUND).astype(np.int16)
                c16[pos % 16, cj * ST + pos // 16] = (cc2 - (g & 1) * BOUND).astype(np.int16)
                s16[pos % 16, cj * ST + pos // 16] = dd.astype(np.int16)
                ecol = cj * CT + pos
                ed[:, ecol] = e[ee].T.astype(nbf16)
                eid[ecol] = ee
            Lr.append(_wrap_idx(r16))
            Lc.append(_wrap_idx(c16))
            Ls.append(_wrap_idx(s16))
            Edat.append(ed)
            Eids.append(eid)
        return dict(NB=NBt, bases=bases, r=Lr, c=Lc, s=Ls, e=Edat, eids=Eids)

    CL = build_tie(True)
    RL = build_tie(False)

    # ---------------- node-slice arrays ----------------
    NSP = NTILE * 128
    dinv_nm, cpl_nm, sden_fm, h_sl = [], [], [], []
    for k in range(M):
        sl = slice(k * NS, (k + 1) * NS)
        di = np.ones(NSP, np.float32); di[:NS] = deg_inv[sl]
        cm = np.zeros(NSP, np.float32); cm[:NS] = is_cpl[sl].astype(np.float32)
        sd = np.zeros(NSP, np.float32); sd[:NS] = sden[sl]
        dinv_nm.append(di.reshape(NTILE, 128).T.copy())     # [128, NTILE]
        cpl_nm.append(cm.reshape(NTILE, 128).T.copy())
        sden_fm.append(sd[None, :].astype(nbf16))            # [1, NSP]
        h_sl.append(h[sl].astype(nbf16))

    plan = dict(
        N=N, E=E, M=M, NS=NS, CA=CA, CT=CT, NA=NA, a_bases=a_bases,
        a_groups=a_groups, NTILE=NTILE, NSP=NSP,
        A_g=A_g, A_w=A_w, A_d=A_d, CL=CL, RL=RL,
        dinv=dinv_nm, cpl=cpl_nm, sden=sden_fm, h_sl=h_sl,
        is_cpl=is_cpl,
        h_bf=h.astype(nbf16),
    )
    return plan


# ----------------------------------------------------------------------------
# device graph
# ----------------------------------------------------------------------------

def _build(plan, weights):
    import concourse.bass as bass
    import concourse.bacc as bacc
    import concourse.mybir as mybir
    from concourse.tile import TileContext
    from concourse import library_config

    BF16 = mybir.dt.bfloat16
    F32 = mybir.dt.float32
    I16 = mybir.dt.int16
    AF = mybir.ActivationFunctionType

    N = plan['N']; NS = plan['NS']; M = plan['M']
    CA = plan['CA']; CT = plan['CT']; NA = plan['NA']
    NTILE = plan['NTILE']
    SROWS = NS + 16
    CLn = plan['CL']['NB']; RLn = plan['RL']['NB']
    bg2_val = float(weights['b_g2'][0])

    nc = bacc.Bacc(None, target_bir_lowering=False, num_devices=M)

    P = {}
    def inp(name, shape, dt):
        P[name] = nc.declare_dram_parameter(name, list(shape), dt, isOutput=False)
        return P[name]

    h0 = inp('h0', [N, H], BF16)
    hsl0 = inp('hsl0', [NS, H], BF16)
    NGT = NA * (CA // 128)
    ag = inp('A_g', [128, NA * (CA // 16)], I16) if NA else None
    aw = inp('A_w', [128, NGT], F32) if NA else None
    ad = inp('A_d', [128, NGT], F32) if NA else None
    iotap = inp('iota', [128, 128], BF16)
    clr = inp('CL_r', [128, CLn * (CT // 16)], I16) if CLn else None
    clc = inp('CL_c', [128, CLn * (CT // 16)], I16) if CLn else None
    cls = inp('CL_s', [128, CLn * (CT // 16)], I16) if CLn else None
    cle = inp('CL_e', [128, CLn * CT], BF16) if CLn else None
    rlr = inp('RL_r', [128, RLn * (CT // 16)], I16) if RLn else None
    rlc = inp('RL_c', [128, RLn * (CT // 16)], I16) if RLn else None
    rls = inp('RL_s', [128, RLn * (CT // 16)], I16) if RLn else None
    rle = inp('RL_e', [128, RLn * CT], BF16) if RLn else None
    dinv = inp('dinv', [128, NTILE], F32)
    cplm = inp('cpl', [128, NTILE], F32)
    sdenp = inp('sden', [1, plan['NSP']], BF16)
    wer1 = inp('Wer1', [384, 256], BF16)
    wer2 = inp('Wer2', [256, 128], BF16)
    wnn = inp('Wnn', [128, 128], BF16)
    bnnp = inp('bnn', [1, 128], BF16)
    wg1 = inp('Wg1', [384, 128], BF16)
    wg2 = inp('Wg2', [128, 1], BF16)
    wiht = inp('WihT', [128, 384], BF16)
    whht = inp('WhhT', [128, 384], BF16)
    ber1 = inp('ber1', [128, 2], F32)
    ber2 = inp('ber2', [128, 1], F32)
    bg1 = inp('bg1', [128, 1], F32)
    bg2p = inp('bg2', [1, 1], F32)
    brz = inp('brz', [128, 2], F32)
    bin_ = inp('bin', [128, 1], F32)
    bhn = inp('bhn', [128, 1], F32)
    identp = inp('ident', [128, 128], BF16)
    onesp = inp('ones1', [1, 128], BF16)
    # zero-initialized tie-scatter accumulators (device zeroes internal DRAM)
    mebufs = [nc.dram_tensor(f'me{it}', [SROWS, H], BF16) for it in range(N_ITER)]

    h_out = nc.declare_dram_parameter('h_out', [NS, H], F32, isOutput=True)
    e_out = (nc.declare_dram_parameter('e_out', [128, CLn * CT], F32, isOutput=True)
             if CLn else None)

    hsl_new = nc.dram_tensor('hsl_new', [NS, H], BF16)
    h_work2 = nc.dram_tensor('h_work2', [N, H], BF16, addr_space='Shared')

    with TileContext(nc) as tc:
        nc.gpsimd.load_library(library_config.mlp)
        with tc.tile_pool(name='res', bufs=1) as res, \
             tc.tile_pool(name='agp', bufs=2) as agp, \
             tc.tile_pool(name='tie', bufs=2) as tiep, \
             tc.tile_pool(name='node', bufs=2) as nodep, \
             tc.tile_pool(name='hold', bufs=10) as holdp, \
             tc.tile_pool(name='ps', bufs=1, space='PSUM') as ps:
            pst = ps

            def load_res(pname, shape, dt, tag):
                t = res.tile(list(shape), dt, tag=tag)
                nc.sync.dma_start(t[...], P[pname][...])
                return t

            # zero the tie-scatter accumulators
            ZC = 2048
            zt = res.tile([128, ZC], BF16, tag='zt')
            nc.vector.memset(zt[...], 0.0)
            for tgt in mebufs:
                for r0 in range(0, SROWS, ZC):
                    nr = min(ZC, SROWS - r0)
                    nc.sync.dma_start(tgt[r0:r0 + nr, :], zt[:, 0:nr])

            # resident loads
            ag_sb = load_res('A_g', [128, NA * (CA // 16)], I16, 'ag') if NA else None
            aw_sb = load_res('A_w', [128, NGT], F32, 'aw') if NA else None
            ad_sb = load_res('A_d', [128, NGT], F32, 'ad') if NA else None
            iota_sb = load_res('iota', [128, 128], BF16, 'iota')
            S_sb = [res.tile([128, 128], BF16, tag=f'Ssb{t}', name=f'Ssb{t}') for t in range(NTILE)]
            tiedat = {}
            for nm, nb in (('CL', CLn), ('RL', RLn)):
                if nb == 0:
                    continue
                tiedat[nm] = dict(
                    r=load_res(nm + '_r', [128, nb * (CT // 16)], I16, nm + 'r'),
                    c=load_res(nm + '_c', [128, nb * (CT // 16)], I16, nm + 'c'),
                    s=load_res(nm + '_s', [128, nb * (CT // 16)], I16, nm + 's'),
                    e=load_res(nm + '_e', [128, nb * CT], BF16, nm + 'e'),
                )
            dinv_sb = load_res('dinv', [128, NTILE], F32, 'dinv')
            cpl_sb = load_res('cpl', [128, NTILE], F32, 'cpl')
            sden_sb = load_res('sden', [1, plan['NSP']], BF16, 'sden')
            ident_sb = load_res('ident', [128, 128], BF16, 'ident')
            ones_sb = load_res('ones1', [1, 128], BF16, 'ones')
            wnn_sb = load_res('Wnn', [128, 128], BF16, 'wnn')
            bnn_sb = load_res('bnn', [1, 128], BF16, 'bnn')
            wg2_sb = load_res('Wg2', [128, 1], BF16, 'wg2')
            wiht_sb = load_res('WihT', [128, 384], BF16, 'wiht')
            whht_sb = load_res('WhhT', [128, 384], BF16, 'whht')
            ber1_sb = load_res('ber1', [128, 2], F32, 'ber1')
            ber2_sb = load_res('ber2', [128, 1], F32, 'ber2')
            bg1_sb = load_res('bg1', [128, 1], F32, 'bg1')
            bg2_sb = load_res('bg2', [1, 1], F32, 'bg2')
            brz_sb = load_res('brz', [128, 2], F32, 'brz')
            bin_sb = load_res('bin', [128, 1], F32, 'bin')
            bhn_sb = load_res('bhn', [128, 1], F32, 'bhn')

            w1k = []
            for kk in range(3):
                t = res.tile([128, 256], BF16, tag=f'w1k{kk}')
                nc.sync.dma_start(t[...], P['Wer1'][kk * 128:(kk + 1) * 128, :])
                w1k.append(t)
            w2k = []
            for kk in range(2):
                t = res.tile([128, 128], BF16, tag=f'w2k{kk}')
                nc.sync.dma_start(t[...], P['Wer2'][kk * 128:(kk + 1) * 128, :])
                w2k.append(t)
            wg1k = []
            for kk in range(3):
                t = res.tile([128, 128], BF16, tag=f'wg1k{kk}')
                nc.sync.dma_start(t[...], P['Wg1'][kk * 128:(kk + 1) * 128, :])
                wg1k.append(t)

            e_res = {}
            for nm, nb in (('CL', CLn), ('RL', RLn)):
                if nb:
                    e_res[nm] = tiedat[nm]['e']

            NCG = CA // 128   # col-groups per A chunk
            NTG = CT // 128

            def tie_phase(nm, nb, bases, h_src, it):
                if nb == 0:
                    return
                td = tiedat[nm]
                e_sb = td['e']
                for j in range(nb):
                    rb, cb = bases[j]
                    hr = tiep.tile([128, 1, CT], BF16, tag='hr')
                    nc.gpsimd.dma_gather(
                        hr[...], h_src[rb:N, :], td['r'][:, j * (CT // 16):(j + 1) * (CT // 16)],
                        CT, CT, H, transpose=True)
                    hc = tiep.tile([128, 1, CT], BF16, tag='hc')
                    nc.gpsimd.dma_gather(
                        hc[...], h_src[cb:N, :], td['c'][:, j * (CT // 16):(j + 1) * (CT // 16)],
                        CT, CT, H, transpose=True)
                    ecol = e_sb[:, j * CT:(j + 1) * CT]
                    p1a = pst.tile([128, CT], F32, tag='pA')
                    p1b = pst.tile([128, CT], F32, tag='pB')
                    for half, (pp, wslice) in enumerate(((p1a, slice(0, 128)), (p1b, slice(128, 256)))):
                        nc.tensor.matmul(pp[...], w1k[0][:, wslice], ecol, start=True, stop=False)
                        nc.tensor.matmul(pp[...], w1k[1][:, wslice], hr[:, 0, :], start=False, stop=False)
                        nc.tensor.matmul(pp[...], w1k[2][:, wslice], hc[:, 0, :], start=False, stop=True)
                    r1a = tiep.tile([128, CT], BF16, tag='r1a')
                    nc.scalar.activation(r1a[...], p1a[...], AF.Relu, bias=ber1_sb[:, 0:1])
                    r1b = tiep.tile([128, CT], BF16, tag='r1b')
                    nc.scalar.activation(r1b[...], p1b[...], AF.Relu, bias=ber1_sb[:, 1:2])
                    p2 = pst.tile([128, CT], F32, tag='pC')
                    nc.tensor.matmul(p2[...], w2k[0][...], r1a[...], start=True, stop=False)
                    nc.tensor.matmul(p2[...], w2k[1][...], r1b[...], start=False, stop=True)
                    # e += p2 + ber2
                    tmp = tiep.tile([128, CT], BF16, tag='etmp')
                    nc.vector.tensor_scalar_add(tmp[...], p2[...], ber2_sb[:, 0:1])
                    nc.vector.tensor_add(ecol, ecol, tmp[...])
                    # transpose to edge-major and scatter into M_e
                    st = tiep.tile([128, NTG, 128], BF16, tag='st')
                    for bb in range(NTG):
                        pt = pst.tile([128, 128], BF16, tag='pT')
                        nc.tensor.transpose(pt[...], ecol[:, bb * 128:(bb + 1) * 128], ident_sb[...])
                        nc.vector.tensor_copy(st[:, bb, :], pt[...])
                    nc.gpsimd.dma_scatter_add(
                        mebufs[it][...], st[...], td['s'][:, j * (CT // 16):(j + 1) * (CT // 16)],
                        CT, CT, H)

            def a_phase(h_src, it):
                accs = {}
                for j in range(NA):
                    base = plan['a_bases'][j]
                    g = agp.tile([128, NCG, 128], BF16, tag='gath')
                    nc.gpsimd.dma_gather(
                        g[...], h_src[base:N, :], ag_sb[:, j * (CA // 16):(j + 1) * (CA // 16)],
                        CA, CA, H, single_packet=(CA <= 1024))
                    for gg in range(NCG):
                        gi = j * NCG + gg
                        dt, gstart, gstop, firstpass = plan['a_groups'][gi]
                        oh = agp.tile([128, 128], BF16, tag='oh')
                        nc.vector.tensor_scalar(
                            oh[...], iota_sb[...], ad_sb[:, gi:gi + 1], aw_sb[:, gi:gi + 1],
                            mybir.AluOpType.is_equal, mybir.AluOpType.mult)
                        if gstart:
                            accs[dt] = ps.tile([128, 128], F32, name=f'acc{dt}',
                                               tag='pD' if dt % 2 == 0 else 'pE')
                        nc.tensor.matmul(accs[dt][...], oh[...], g[:, gg, :],
                                         start=gstart, stop=gstop)
                        if gstop:
                            if firstpass:
                                nc.vector.tensor_copy(S_sb[dt][...], accs[dt][...])
                            else:
                                nc.vector.tensor_add(S_sb[dt][...], S_sb[dt][...], accs[dt][...])

            def node_phase(h_old_src, it):
                # groups of up to 4 node tiles (free dim <= 512)
                t0 = 0
                while t0 < NTILE:
                    nt = min(4, NTILE - t0)
                    F = nt * 128
                    S_T = nodep.tile([128, 512], BF16, tag='S_T')
                    Me_T = nodep.tile([128, 512], BF16, tag='Me_T')
                    H_T = nodep.tile([128, 512], BF16, tag='H_T')
                    hots = []
                    for u in range(nt):
                        r0 = (t0 + u) * 128
                        r1 = min(r0 + 128, NS)
                        nr = r1 - r0
                        ptn = pst.tile([128, 128], BF16, tag='pT')
                        nc.tensor.transpose(ptn[...], S_sb[t0 + u][...], ident_sb[...])
                        nc.vector.tensor_copy(S_T[:, u * 128:(u + 1) * 128], ptn[...])
                        met = nodep.tile([128, 128], BF16, tag='met')
                        nc.sync.dma_start(met[:nr, :], mebufs[it][r0:r1, :])
                        nc.vector.tensor_scalar_mul(met[...], met[...], dinv_sb[:, t0 + u:t0 + u + 1])
                        ptn2 = pst.tile([128, 128], BF16, tag='pT')
                        nc.tensor.transpose(ptn2[...], met[...], ident_sb[...])
                        nc.vector.tensor_copy(Me_T[:, u * 128:(u + 1) * 128], ptn2[...])
                        hot = holdp.tile([128, 128], BF16, tag='hot')
                        nc.sync.dma_start(hot[:nr, :], h_old_src[r0:r1, :])
                        ptn3 = pst.tile([128, 128], BF16, tag='pT')
                        nc.tensor.transpose(ptn3[...], hot[...], ident_sb[...])
                        nc.vector.tensor_copy(H_T[:, u * 128:(u + 1) * 128], ptn3[...])
                        hots.append((hot, r0, r1))
                    STf = S_T[:, 0:F]; MeTf = Me_T[:, 0:F]; HTf = H_T[:, 0:F]
                    nbase = t0 * 128
                    pmn = ps.tile([128, 512], F32, tag='pA')
                    nc.tensor.matmul(pmn[:, 0:F], wnn_sb[...], STf, start=True, stop=False)
                    nc.tensor.matmul(pmn[:, 0:F], bnn_sb[...],
                                     sden_sb[0:1, nbase:nbase + F], start=False, stop=True)
                    mn_T = nodep.tile([128, 512], BF16, tag='mn_T')
                    nc.vector.tensor_copy(mn_T[:, 0:F], pmn[:, 0:F])
                    pg1 = ps.tile([128, 512], F32, tag='pB')
                    nc.tensor.matmul(pg1[:, 0:F], wg1k[0][...], MeTf, start=True, stop=False)
                    nc.tensor.matmul(pg1[:, 0:F], wg1k[1][...], mn_T[:, 0:F], start=False, stop=False)
                    nc.tensor.matmul(pg1[:, 0:F], wg1k[2][...], HTf, start=False, stop=True)
                    rg = nodep.tile([128, 512], BF16, tag='rg')
                    nc.scalar.activation(rg[:, 0:F], pg1[:, 0:F], AF.Relu, bias=bg1_sb[:, 0:1])
                    pg2 = ps.tile([1, 512], F32, tag='pC')
                    nc.tensor.matmul(pg2[:, 0:F], wg2_sb[...], rg[:, 0:F], start=True, stop=True)
                    gate = nodep.tile([1, 512], BF16, tag='gate')
                    nc.scalar.activation(gate[:, 0:F], pg2[:, 0:F], AF.Sigmoid, bias=bg2_sb[:, 0:1])
                    pgb = ps.tile([128, 512], F32, tag='pC')
                    nc.tensor.matmul(pgb[:, 0:F], ones_sb[...], gate[:, 0:F], start=True, stop=True)
                    m_T = nodep.tile([128, 512], BF16, tag='m_T')
                    nc.vector.tensor_mul(m_T[:, 0:F], pgb[:, 0:F], mn_T[:, 0:F])
                    nc.vector.tensor_add(m_T[:, 0:F], m_T[:, 0:F], MeTf)
                    # GRU
                    pr = ps.tile([128, 512], F32, tag='pD')
                    nc.tensor.matmul(pr[:, 0:F], wiht_sb[:, 0:128], m_T[:, 0:F], start=True, stop=False)
                    nc.tensor.matmul(pr[:, 0:F], whht_sb[:, 0:128], HTf, start=False, stop=True)
                    pz = ps.tile([128, 512], F32, tag='pE')
                    nc.tensor.matmul(pz[:, 0:F], wiht_sb[:, 128:256], m_T[:, 0:F], start=True, stop=False)
                    nc.tensor.matmul(pz[:, 0:F], whht_sb[:, 128:256], HTf, start=False, stop=True)
                    pgin = ps.tile([128, 512], F32, tag='pF')
                    nc.tensor.matmul(pgin[:, 0:F], wiht_sb[:, 256:384], m_T[:, 0:F], start=True, stop=True)
                    pghn = ps.tile([128, 512], F32, tag='pG')
                    nc.tensor.matmul(pghn[:, 0:F], whht_sb[:, 256:384], HTf, start=True, stop=True)
                    rr = nodep.tile([128, 512], F32, tag='rr')
                    nc.scalar.activation(rr[:, 0:F], pr[:, 0:F], AF.Sigmoid, bias=brz_sb[:, 0:1])
                    zz = nodep.tile([128, 512], F32, tag='zz')
                    nc.scalar.activation(zz[:, 0:F], pz[:, 0:F], AF.Sigmoid, bias=brz_sb[:, 1:2])
                    t1 = nodep.tile([128, 512], F32, tag='t1')
                    nc.vector.tensor_scalar_add(t1[:, 0:F], pghn[:, 0:F], bhn_sb[:, 0:1])
                    nc.vector.tensor_mul(t1[:, 0:F], t1[:, 0:F], rr[:, 0:F])
                    nc.vector.tensor_add(t1[:, 0:F], t1[:, 0:F], pgin[:, 0:F])
                    nn_ = nodep.tile([128, 512], F32, tag='nn_')
                    nc.scalar.activation(nn_[:, 0:F], t1[:, 0:F], AF.Tanh, bias=bin_sb[:, 0:1])
                    # h_new = n + z*(h - n)
                    d = nodep.tile([128, 512], BF16, tag='d')
                    nc.vector.tensor_sub(d[:, 0:F], HTf, nn_[:, 0:F])
                    nc.vector.tensor_mul(d[:, 0:F], d[:, 0:F], zz[:, 0:F])
                    nc.vector.tensor_add(d[:, 0:F], d[:, 0:F], nn_[:, 0:F])
                    for u in range(nt):
                        hot, r0, r1 = hots[u]
                        nr = r1 - r0
                        pt2 = pst.tile([128, 128], BF16, tag='pT')
                        nc.tensor.transpose(pt2[...], d[:, u * 128:(u + 1) * 128], ident_sb[...])
                        hn = nodep.tile([128, 128], BF16, tag='hn')
                        nc.vector.tensor_sub(hn[...], pt2[...], hot[...])
                        nc.vector.tensor_scalar_mul(hn[...], hn[...], cpl_sb[:, t0 + u:t0 + u + 1])
                        nc.vector.tensor_add(hn[...], hn[...], hot[...])
                        if it == 0:
                            nc.sync.dma_start(hsl_new[r0:r1, :], hn[:nr, :])
                        else:
                            nc.gpsimd.dma_start(h_out[r0:r1, :], hn[:nr, :])
                    t0 += nt

            import concourse.mybir as mybir2
            for it in range(N_ITER):
                h_src = h0 if it == 0 else h_work2
                h_old_src = hsl0 if it == 0 else hsl_new
                tie_phase('CL', CLn, plan['CL']['bases'], h_src, it)
                tie_phase('RL', RLn, plan['RL']['bases'], h_src, it)
                if NA:
                    a_phase(h_src, it)
                node_phase(h_old_src, it)
                if it == 0:
                    nc.gpsimd.collective_compute(
                        'AllGather', mybir2.AluOpType.bypass,
                        replica_groups=[list(range(M))],
                        ins=[hsl_new[:, :]], outs=[h_work2[:, :]])
            if CLn:
                nc.gpsimd.dma_start(e_out[:, :], e_res['CL'][...])

    nc.compile()
    return nc


# ----------------------------------------------------------------------------
# run + assemble
# ----------------------------------------------------------------------------

def _make_in_maps(plan, weights):
    M = plan['M']
    CLn = plan['CL']['NB']; RLn = plan['RL']['NB']
    shared = dict(
        Wer1=weights['W_er1'].astype(nbf16),
        Wer2=weights['W_er2'].astype(nbf16),
        Wnn=weights['W_nn'].astype(nbf16),
        bnn=weights['b_nn'][None, :].astype(nbf16),
        Wg1=weights['W_g1'].astype(nbf16),
        Wg2=weights['W_g2'].astype(nbf16),
        WihT=weights['W_ih'].T.copy().astype(nbf16),
        WhhT=weights['W_hh'].T.copy().astype(nbf16),
        ber1=weights['b_er1'].reshape(2, 128).T.copy().astype(np.float32),
        ber2=weights['b_er2'][:, None].astype(np.float32),
        bg1=weights['b_g1'][:, None].astype(np.float32),
        bg2=weights['b_g2'].reshape(1, 1).astype(np.float32),
        brz=(weights['b_ih'] + weights['b_hh'])[:256].reshape(2, 128).T.copy().astype(np.float32),
        bin=weights['b_ih'][256:384][:, None].astype(np.float32),
        bhn=weights['b_hh'][256:384][:, None].astype(np.float32),
        ident=np.eye(128, dtype=np.float32).astype(nbf16),
        iota=np.tile(np.arange(128, dtype=np.float32)[None, :], (128, 1)).astype(nbf16),
        ones1=np.ones((1, 128), np.float32).astype(nbf16),
    )
    in_maps = []
    for k in range(M):
        m = dict(shared)
        m['h0'] = plan['h_bf']
        m['hsl0'] = plan['h_sl'][k]
        if plan['NA']:
            m['A_g'] = plan['A_g'][k]; m['A_w'] = plan['A_w'][k]; m['A_d'] = plan['A_d'][k]
        if CLn:
            m['CL_r'] = plan['CL']['r'][k]; m['CL_c'] = plan['CL']['c'][k]
            m['CL_s'] = plan['CL']['s'][k]; m['CL_e'] = plan['CL']['e'][k]
        if RLn:
            m['RL_r'] = plan['RL']['r'][k]; m['RL_c'] = plan['RL']['c'][k]
            m['RL_s'] = plan['RL']['s'][k]; m['RL_e'] = plan['RL']['e'][k]
        m['dinv'] = plan['dinv'][k]; m['cpl'] = plan['cpl'][k]; m['sden'] = plan['sden'][k]
        in_maps.append(m)
    return in_maps


def _assemble(plan, h, e, results):
    M = plan['M']; NS = plan['NS']
    h_out = np.concatenate([np.asarray(results[k]['h_out']) for k in range(M)], axis=0)
    h_out = h_out.astype(np.float32)
    h_out[~plan['is_cpl']] = h[~plan['is_cpl']]
    e_out = e.astype(np.float32).copy()
    if plan['CL']['NB']:
        for k in range(M):
            eo = np.asarray(results[k]['e_out'])      # [128, CLn*CT] f32
            eids = plan['CL']['eids'][k]
            vmask = eids >= 0
            e_out[eids[vmask]] = eo[:, vmask].T
    return h_out, e_out


def _run(plan, weights, use_sim=False, trace=False):
    nc = _build(plan, weights)
    in_maps = _make_in_maps(plan, weights)
    M = plan['M']
    if use_sim:
        from concourse import bass_interp
        sim = bass_interp.MultiCoreSim(nc, M)
        for k in range(M):
            for name, arr in in_maps[k].items():
                sim.cores[k].tensor(name)[:] = arr
        sim.simulate()
        results = []
        for k in range(M):
            r = {'h_out': np.array(sim.cores[k].tensor('h_out')[:])}
            if plan['CL']['NB']:
                r['e_out'] = np.array(sim.cores[k].tensor('e_out')[:])
            results.append(r)
        return results, None
    else:
        from concourse.bass_utils import run_bass_kernel_spmd
        out = run_bass_kernel_spmd(nc, in_maps, core_ids=list(range(M)), trace=trace)
        return out.results, out


def kernel(h, e, edge_attr_raw,
           W_er1, b_er1, W_er2, b_er2,
           W_ih, W_hh, b_ih, b_hh,
           W_nn, b_nn,
           W_g1, b_g1, W_g2, b_g2,
           edge_index, is_tie):
    h = np.asarray(h, np.float32)
    e = np.asarray(e, np.float32)
    weights = dict(W_er1=np.asarray(W_er1, np.float32), b_er1=np.asarray(b_er1, np.float32),
                   W_er2=np.asarray(W_er2, np.float32), b_er2=np.asarray(b_er2, np.float32),
                   W_ih=np.asarray(W_ih, np.float32), W_hh=np.asarray(W_hh, np.float32),
                   b_ih=np.asarray(b_ih, np.float32), b_hh=np.asarray(b_hh, np.float32),
                   W_nn=np.asarray(W_nn, np.float32), b_nn=np.asarray(b_nn, np.float32),
                   W_g1=np.asarray(W_g1, np.float32), b_g1=np.asarray(b_g1, np.float32),
                   W_g2=np.asarray(W_g2, np.float32), b_g2=np.asarray(b_g2, np.float32))
    plan = _plan(h, e, np.asarray(edge_attr_raw, np.float32),
                 np.asarray(edge_index), np.asarray(is_tie), M_CORES, CA_DEF, CT_DEF)
    results, _ = _run(plan, weights, use_sim=False)
    h_out, e_out = _assemble(plan, h, e, results)
    return h_out, e_out


# revision 26
# speedup vs baseline: 1.8645x; 1.8645x over previous
"""BCGNN (nn_BCGNN_15934328668763) Trainium2 Bass kernel, 8 NeuronCores.

Destination-sharded edge-parallel design: core k owns node slice
[k*NS, (k+1)*NS). Every segment-sum lands entirely in the owner's slice, so
no all-reduce of [N,H] partials is needed -- only one AllGather of the
updated h between the two refinement iterations.

Sparsity exploited (vs the dense reference):
  - e_work only changes on tie edges (~5% of E): refine MLP runs on those.
  - m_node_all only takes edges with w_norm != 0 (internal & is_cpl[col]).
  - all [E]/[N] scalar index prep (w_norm, deg, is_cpl, sden) is host-side
    sharding logic; all [*,H] tensor compute runs on device in bf16.
"""
import sys, os

for _p in ('/opt/trn_rl_repo', '/root/.axon_site/_ro/trn_rl_repo'):
    if os.path.isdir(_p) and _p not in sys.path:
        sys.path.insert(0, _p)

import numpy as np
import ml_dtypes

nbf16 = ml_dtypes.bfloat16

# problem constants (hardcoded; harness calls kernel() with exactly these shapes)
N_FULL = 50000
E_FULL = 800000
H = 128
M_CORES = 8
N_ITER = 2
BOUND = 32768          # int16 gather index limit (exclusive)

CA_DEF = 2048          # A-stream (m_node) chunk: indices per dma_gather
CT_DEF = 512           # tie-stream chunk


# ----------------------------------------------------------------------------
# host-side planning
# ----------------------------------------------------------------------------

def _wrap_idx(mat16):
    """[16, L] channel-wrapped int16 -> [128, L] replicated for 8 Q7 cores."""
    return np.tile(mat16, (8, 1)).astype(np.int16)


def _plan(h, e, edge_attr_raw, edge_index, is_tie, M, CA, CT):
    N, Hd = h.shape
    E = e.shape[0]
    assert Hd == H
    NS = N // M
    SA = CA // 16           # slots per channel per A-chunk
    ST = CT // 16
    row = edge_index[0].astype(np.int64)
    col = edge_index[1].astype(np.int64)
    tie = np.asarray(is_tie).astype(bool)
    tie_f = tie.astype(np.float32)

    cpl_cnt = (np.bincount(row, weights=tie_f, minlength=N)
               + np.bincount(col, weights=tie_f, minlength=N)).astype(np.float32)
    is_cpl = cpl_cnt > 0
    internal = ~tie
    to_cpl = internal & is_cpl[col]
    X = np.abs(edge_attr_raw[:, 1].astype(np.float32))
    w = np.where(to_cpl, (1.0 / np.sqrt(X * X + np.float32(1e-6))).astype(np.float32),
                 np.float32(0.0)).astype(np.float32)
    w_den = np.bincount(col, weights=w, minlength=N).astype(np.float32)
    w_norm = (w / (w_den[col] + np.float32(1e-6))).astype(np.float32)
    deg = np.maximum(cpl_cnt, np.float32(1.0))
    deg_inv = (np.float32(1.0) / deg).astype(np.float32)
    sden = np.bincount(col, weights=w_norm, minlength=N).astype(np.float32)

    core_of = lambda x: x // NS

    # ---------------- A stream (m_node): one-hot matmul accumulation ----------
    # Edges sorted by (row-bucket, dest). Each (bucket, dest-tile) run is padded
    # to a multiple of 128; a 128-position group feeds one one-hot matmul into
    # the dest-tile's PSUM accumulator. Group structure unified across cores.
    NTILE = -(-NS // 128)
    a_sel_all = to_cpl
    a_core = core_of(col)
    NCG = CA // 128
    per_core = []
    cnt = np.zeros((M, 2, NTILE), np.int64)
    for k in range(M):
        sel = a_sel_all & (a_core == k)
        r = row[sel]
        dloc = (col[sel] - k * NS).astype(np.int64)
        wv = w_norm[sel]
        bk = (r >= BOUND).astype(np.int64)
        order = np.lexsort((dloc, bk))
        r, dloc, wv, bk = r[order], dloc[order], wv[order], bk[order]
        dt = dloc // 128
        for b in (0, 1):
            cnt[k, b] = np.bincount(dt[bk == b], minlength=NTILE)
        per_core.append((r, dloc, wv, bk, dt))
    G = np.maximum(-(-cnt.max(axis=0) // 128), 0)        # [2, NTILE] groups per run
    # pad each bucket's group count to a chunk multiple (merge pad into last run)
    for b in (0, 1):
        ng = int(G[b].sum())
        if ng == 0:
            continue
        padg = (-ng) % NCG
        last = np.nonzero(G[b])[0][-1]
        G[b, last] += padg
    NA = int(G.sum()) // NCG
    a_bases = []
    for b in (0, 1):
        a_bases += [b * BOUND] * (int(G[b].sum()) // NCG)
    # group metadata: (dt, start, stop, first_pass) per global group
    a_groups = []
    gfirst = {}          # (b, dt) -> first group index
    gi = 0
    for b in (0, 1):
        for dtv in range(NTILE):
            n = int(G[b, dtv])
            if n == 0:
                continue
            gfirst[(b, dtv)] = gi
            firstpass = (b == 0) or (G[0, dtv] == 0)
            for u in range(n):
                a_groups.append((dtv, u == 0, u == n - 1, firstpass))
                gi += 1
    NGT = gi
    assert NGT == NA * NCG

    A_g, A_oh = [], []
    for k in range(M):
        g16 = np.zeros((16, NA * (CA // 16)), np.int16)
        ohmat = np.zeros((128, NGT * 128), nbf16)
        r, dloc, wv, bk, dt = per_core[k]
        run_id = bk * NTILE + dt
        # start position of each run in the unified stream
        run_base = np.zeros(2 * NTILE, np.int64)
        for b in (0, 1):
            for dtv in range(NTILE):
                if (b, dtv) in gfirst:
                    run_base[b * NTILE + dtv] = 128 * gfirst[(b, dtv)]
        # rank within run
        uniq, inv_start = np.unique(run_id, return_index=True)
        starts = np.zeros(2 * NTILE, np.int64)
        starts[uniq] = inv_start
        rank = np.arange(len(run_id)) - starts[run_id]
        pos = run_base[run_id] + rank
        g16[pos % 16, pos // 16] = (r - bk * BOUND).astype(np.int16)
        ohmat[pos % 128, (pos // 128) * 128 + (dloc - dt * 128)] = wv.astype(nbf16)
        A_g.append(_wrap_idx(g16))
        A_oh.append(ohmat)

    # ---------------- merged tie stream ----------------
    # Core k refines every tie edge whose col OR row lands in its slice (one
    # refine per edge). Each window is [both | col-only | row-only] segments
    # (each padded to 128). Scatter calls: col-dests over [both+col-only];
    # row-dests over [both] and over [row-only]. Greedy window assignment
    # keeps col-dests and row-dests duplicate-free inside every window.
    tidx_all = np.nonzero(tie)[0]
    CSEG = 1024          # per-window capacity: col-owned entries
    RSEG = 1024          # row-owned entries

    def build_tie_merged():
        # Entries: (core, seg): seg 0 = col-owned (dest=col, global endpoint=row),
        # seg 1 = row-owned (dest=row, global endpoint=col). Edges owned through
        # both endpoints appear once in each segment (refine is recomputed; the
        # two e copies evolve identically). The local endpoint (always inside
        # the owner slice) is gathered from the core's own h slice, so only the
        # global endpoint needs the int16 bucket split -> 2 window groups.
        per_core = []
        for k in range(M):
            ents = []        # per entry: (gsrc, lsrc, dest, eid, seg)
            for seg, own, gcol in ((0, col, row), (1, row, col)):
                sel = (own[tidx_all] // NS) == k
                te = tidx_all[sel]
                ents.append((gcol[te], own[te] - k * NS, own[te] - k * NS, te, seg))
            per_core.append(ents)
        # greedy window packing per bucket group of the global endpoint
        nwin = [0, 0]
        slots = [[None, None] for _ in range(M)]
        demand = {}
        for k in range(M):
            for seg in (0, 1):
                gsrc, lsrc, dest, te, _ = per_core[k][seg]
                bk = (gsrc >= BOUND).astype(np.int64)
                cap = CSEG if seg == 0 else RSEG
                out = np.zeros((len(te), 2), np.int64)
                state = {0: ([], []), 1: ([], [])}     # bucket -> (fills, used-sets)
                for i in range(len(te)):
                    fills, useds = state[bk[i]]
                    w = 0
                    while True:
                        if w == len(fills):
                            fills.append(0)
                            useds.append(set())
                        if fills[w] < cap and dest[i] not in useds[w]:
                            out[i] = (bk[i] * 1000 + w, fills[w])
                            fills[w] += 1
                            useds[w].add(dest[i])
                            break
                        w += 1
                for b in (0, 1):
                    fills, _ = state[b]
                    nwin[b] = max(nwin[b], len(fills))
                    for w, f in enumerate(fills):
                        key = (b, w, seg)
                        demand[key] = max(demand.get(key, 0), f)
                slots[k][seg] = out
        r128 = lambda x: -(-x // 128) * 128
        win_meta = []        # (bucket, off, Lc, Lr)
        off = 0
        offmap = {}
        for b in (0, 1):
            for w in range(nwin[b]):
                Lc = r128(demand.get((b, w, 0), 0))
                Lr = r128(demand.get((b, w, 1), 0))
                if Lc + Lr == 0:
                    continue
                win_meta.append((b, off, Lc, Lr))
                offmap[(b, w)] = (off, Lc, Lr)
                off += Lc + Lr
        TOT = off
        Gg, Gl, Ts, Edat, Eids = [], [], [], [], []
        for k in range(M):
            g16 = np.zeros((16, TOT // 16), np.int16)
            l16 = np.zeros((16, TOT // 16), np.int16)
            s16 = np.full((16, TOT // 16), NS, np.int16)
            ed = np.zeros((128, TOT), nbf16)
            eid = np.full((TOT,), -1, np.int64)
            for seg in (0, 1):
                gsrc, lsrc, dest, te, _ = per_core[k][seg]
                sl = slots[k][seg]
                if len(te) == 0:
                    continue
                b = sl[:, 0] // 1000
                w = sl[:, 0] % 1000
                base = np.array([offmap[(bb, ww)][0] + (0 if seg == 0 else offmap[(bb, ww)][1])
                                 for bb, ww in zip(b, w)], np.int64)
                pos = base + sl[:, 1]
                g16[pos % 16, pos // 16] = (gsrc - b * BOUND).astype(np.int16)
                l16[pos % 16, pos // 16] = lsrc.astype(np.int16)
                s16[pos % 16, pos // 16] = dest.astype(np.int16)
                ed[:, pos] = e[te].T.astype(nbf16)
                if seg == 0:
                    eid[pos] = te
            Gg.append(_wrap_idx(g16))
            Gl.append(_wrap_idx(l16))
            Ts.append(_wrap_idx(s16))
            Edat.append(ed)
            Eids.append(eid)
        return dict(TOT=TOT, win=win_meta, g=Gg, l=Gl, s=Ts, e=Edat, eids=Eids)

    TIE = build_tie_merged()

    # ---------------- node-slice arrays ----------------
    NSP = NTILE * 128
    dinv_nm, cpl_nm, sden_fm, h_sl = [], [], [], []
    for k in range(M):
        sl = slice(k * NS, (k + 1) * NS)
        di = np.ones(NSP, np.float32); di[:NS] = deg_inv[sl]
        cm = np.zeros(NSP, np.float32); cm[:NS] = is_cpl[sl].astype(np.float32)
        sd = np.zeros(NSP, np.float32); sd[:NS] = sden[sl]
        dinv_nm.append(di.reshape(NTILE, 128).T.copy())     # [128, NTILE]
        cpl_nm.append(cm.reshape(NTILE, 128).T.copy())
        sden_fm.append(sd[None, :].astype(nbf16))            # [1, NSP]
        h_sl.append(h[sl].astype(nbf16))

    plan = dict(
        N=N, E=E, M=M, NS=NS, CA=CA, CT=CT, NA=NA, a_bases=a_bases,
        a_groups=a_groups, NTILE=NTILE, NSP=NSP, NGT=NGT,
        A_g=A_g, A_oh=A_oh, TIE=TIE,
        dinv=dinv_nm, cpl=cpl_nm, sden=sden_fm, h_sl=h_sl,
        is_cpl=is_cpl,
        h_bf=h.astype(nbf16),
    )
    return plan


# ----------------------------------------------------------------------------
# device graph
# ----------------------------------------------------------------------------

def _build(plan, weights):
    import concourse.bass as bass
    import concourse.bacc as bacc
    import concourse.mybir as mybir
    from concourse.tile import TileContext
    from concourse import library_config

    BF16 = mybir.dt.bfloat16
    F32 = mybir.dt.float32
    I16 = mybir.dt.int16
    AF = mybir.ActivationFunctionType

    N = plan['N']; NS = plan['NS']; M = plan['M']
    CA = plan['CA']; CT = plan['CT']; NA = plan['NA']
    NTILE = plan['NTILE']
    SROWS = NS + 16
    TOT = plan['TIE']['TOT']
    bg2_val = float(weights['b_g2'][0])

    nc = bacc.Bacc(None, target_bir_lowering=False, num_devices=M)

    P = {}
    def inp(name, shape, dt):
        P[name] = nc.declare_dram_parameter(name, list(shape), dt, isOutput=False)
        return P[name]

    h0 = inp('h0', [N, H], BF16)
    hsl0 = inp('hsl0', [NS, H], BF16)
    NGT = plan['NGT']
    ag = inp('A_g', [128, NA * (CA // 16)], I16) if NA else None
    aoh = inp('A_oh', [128, NGT * 128], BF16) if NA else None
    if TOT:
        inp('TG_g', [128, TOT // 16], I16)
        inp('TG_l', [128, TOT // 16], I16)
        inp('TS', [128, TOT // 16], I16)
        inp('TIE_e', [128, TOT], BF16)
    dinv = inp('dinv', [128, NTILE], F32)
    cplm = inp('cpl', [128, NTILE], F32)
    sdenp = inp('sden', [1, plan['NSP']], BF16)
    wer1 = inp('Wer1', [384, 256], BF16)
    wer2 = inp('Wer2', [256, 128], BF16)
    wnn = inp('Wnn', [128, 128], BF16)
    bnnp = inp('bnn', [1, 128], BF16)
    wg1 = inp('Wg1', [384, 128], BF16)
    wg2 = inp('Wg2', [128, 1], BF16)
    wiht = inp('WihT', [128, 384], BF16)
    whht = inp('WhhT', [128, 384], BF16)
    ber1 = inp('ber1', [128, 2], F32)
    ber2 = inp('ber2', [128, 1], F32)
    bg1 = inp('bg1', [128, 1], F32)
    bg2p = inp('bg2', [1, 1], F32)
    brz = inp('brz', [128, 2], F32)
    bin_ = inp('bin', [128, 1], F32)
    bhn = inp('bhn', [128, 1], F32)
    identp = inp('ident', [128, 128], BF16)
    onesp = inp('ones1', [1, 128], BF16)
    # zero-initialized tie-scatter accumulators (device zeroes internal DRAM)
    mebufs = [nc.dram_tensor(f'me{it}', [SROWS, H], BF16) for it in range(N_ITER)]

    h_out = nc.declare_dram_parameter('h_out', [NS, H], F32, isOutput=True)
    e_out = (nc.declare_dram_parameter('e_out', [128, TOT], F32, isOutput=True)
             if TOT else None)

    hsl_new = nc.dram_tensor('hsl_new', [NS, H], BF16)
    h_work2 = nc.dram_tensor('h_work2', [N, H], BF16, addr_space='Shared')

    with TileContext(nc) as tc:
        nc.gpsimd.load_library(library_config.mlp)
        with tc.tile_pool(name='res', bufs=1) as res, \
             tc.tile_pool(name='agp', bufs=2) as agp, \
             tc.tile_pool(name='tie', bufs=2) as tiep, \
             tc.tile_pool(name='node', bufs=2) as nodep, \
             tc.tile_pool(name='hold', bufs=10) as holdp, \
             tc.tile_pool(name='ps', bufs=1, space='PSUM') as ps:
            pst = ps

            def load_res(pname, shape, dt, tag):
                t = res.tile(list(shape), dt, tag=tag)
                nc.sync.dma_start(t[...], P[pname][...])
                return t

            # zero the tie-scatter accumulators
            ZC = 1024
            zt = res.tile([128, ZC], BF16, tag='zt')
            nc.vector.memset(zt[...], 0.0)
            for tgt in mebufs:
                for r0 in range(0, SROWS, ZC):
                    nr = min(ZC, SROWS - r0)
                    nc.sync.dma_start(tgt[r0:r0 + nr, :], zt[:, 0:nr])

            # resident loads
            ag_sb = load_res('A_g', [128, NA * (CA // 16)], I16, 'ag') if NA else None
            S_sb = [res.tile([128, 128], BF16, tag=f'Ssb{t}', name=f'Ssb{t}') for t in range(NTILE)]
            if TOT:
                tg_g = load_res('TG_g', [128, TOT // 16], I16, 'tgg')
                tg_l = load_res('TG_l', [128, TOT // 16], I16, 'tgl')
                ts_sb = load_res('TS', [128, TOT // 16], I16, 'tss')
                e_sb = load_res('TIE_e', [128, TOT], BF16, 'tiee')
            dinv_sb = load_res('dinv', [128, NTILE], F32, 'dinv')
            cpl_sb = load_res('cpl', [128, NTILE], F32, 'cpl')
            sden_sb = load_res('sden', [1, plan['NSP']], BF16, 'sden')
            ident_sb = load_res('ident', [128, 128], BF16, 'ident')
            ones_sb = load_res('ones1', [1, 128], BF16, 'ones')
            wnn_sb = load_res('Wnn', [128, 128], BF16, 'wnn')
            bnn_sb = load_res('bnn', [1, 128], BF16, 'bnn')
            wg2_sb = load_res('Wg2', [128, 1], BF16, 'wg2')
            wiht_sb = load_res('WihT', [128, 384], BF16, 'wiht')
            whht_sb = load_res('WhhT', [128, 384], BF16, 'whht')
            ber1_sb = load_res('ber1', [128, 2], F32, 'ber1')
            ber2_sb = load_res('ber2', [128, 1], F32, 'ber2')
            bg1_sb = load_res('bg1', [128, 1], F32, 'bg1')
            bg2_sb = load_res('bg2', [1, 1], F32, 'bg2')
            brz_sb = load_res('brz', [128, 2], F32, 'brz')
            bin_sb = load_res('bin', [128, 1], F32, 'bin')
            bhn_sb = load_res('bhn', [128, 1], F32, 'bhn')

            w1k = []
            for kk in range(3):
                t = res.tile([128, 256], BF16, tag=f'w1k{kk}')
                nc.sync.dma_start(t[...], P['Wer1'][kk * 128:(kk + 1) * 128, :])
                w1k.append(t)
            w2k = []
            for kk in range(2):
                t = res.tile([128, 128], BF16, tag=f'w2k{kk}')
                nc.sync.dma_start(t[...], P['Wer2'][kk * 128:(kk + 1) * 128, :])
                w2k.append(t)
            wg1k = []
            for kk in range(3):
                t = res.tile([128, 128], BF16, tag=f'wg1k{kk}')
                nc.sync.dma_start(t[...], P['Wg1'][kk * 128:(kk + 1) * 128, :])
                wg1k.append(t)

            NCG = CA // 128   # col-groups per A chunk
            NTG = CT // 128

            def tie_phase(h_src, h_old_src, it):
                for (bb, off, Lc, Lr) in plan['TIE']['win']:
                    W = Lc + Lr
                    g1 = tiep.tile([128, 1, W], BF16, tag='g1', name=f'g1_{it}_{off}')
                    nc.gpsimd.dma_gather(
                        g1[...], h_src[bb * BOUND:N, :], tg_g[:, off // 16:(off + W) // 16],
                        W, W, H, transpose=True, single_packet=(W <= 1024))
                    g2 = tiep.tile([128, 1, W], BF16, tag='g2', name=f'g2_{it}_{off}')
                    nc.gpsimd.dma_gather(
                        g2[...], h_old_src[:, :], tg_l[:, off // 16:(off + W) // 16],
                        W, W, H, transpose=True, single_packet=(W <= 1024))
                    st_c = tiep.tile([128, max(Lc, 128) // 128, 128], BF16, tag='stc',
                                     name=f'stc_{it}_{off}')
                    st_r = tiep.tile([128, max(Lr, 128) // 128, 128], BF16, tag='str',
                                     name=f'str_{it}_{off}')
                    for seg, s0g, Ls in ((0, 0, Lc), (1, Lc, Lr)):
                        for s0 in range(0, Ls, CT):
                            sw = min(CT, Ls - s0)
                            a0 = s0g + s0
                            ecol = e_sb[:, off + a0:off + a0 + sw]
                            hrow = (g1 if seg == 0 else g2)[:, 0, a0:a0 + sw]
                            hcol = (g2 if seg == 0 else g1)[:, 0, a0:a0 + sw]
                            p1a = pst.tile([128, CT], F32, tag='pA')
                            p1b = pst.tile([128, CT], F32, tag='pB')
                            for pp, wsl in ((p1a, slice(0, 128)), (p1b, slice(128, 256))):
                                nc.tensor.matmul(pp[:, 0:sw], w1k[0][:, wsl], ecol, start=True, stop=False)
                                nc.tensor.matmul(pp[:, 0:sw], w1k[1][:, wsl], hrow, start=False, stop=False)
                                nc.tensor.matmul(pp[:, 0:sw], w1k[2][:, wsl], hcol, start=False, stop=True)
                            r1a = tiep.tile([128, CT], BF16, tag='r1a')
                            nc.scalar.activation(r1a[:, 0:sw], p1a[:, 0:sw], AF.Relu, bias=ber1_sb[:, 0:1])
                            r1b = tiep.tile([128, CT], BF16, tag='r1b')
                            nc.scalar.activation(r1b[:, 0:sw], p1b[:, 0:sw], AF.Relu, bias=ber1_sb[:, 1:2])
                            p2 = pst.tile([128, CT], F32, tag='pC')
                            nc.tensor.matmul(p2[:, 0:sw], w2k[0][...], r1a[:, 0:sw], start=True, stop=False)
                            nc.tensor.matmul(p2[:, 0:sw], w2k[1][...], r1b[:, 0:sw], start=False, stop=True)
                            tmp = tiep.tile([128, CT], BF16, tag='etmp')
                            nc.vector.tensor_scalar_add(tmp[:, 0:sw], p2[:, 0:sw], ber2_sb[:, 0:1])
                            nc.vector.tensor_add(ecol, ecol, tmp[:, 0:sw])
                            stt = st_c if seg == 0 else st_r
                            for bb2 in range(s0 // 128, (s0 + sw) // 128):
                                pt = pst.tile([128, 128], BF16, tag='pT')
                                nc.tensor.transpose(pt[...], e_sb[:, off + s0g + bb2 * 128:off + s0g + (bb2 + 1) * 128], ident_sb[...])
                                nc.vector.tensor_copy(stt[:, bb2, :], pt[...])
                    if 'tsc' in os.environ.get('BCGNN_SKIP', ''):
                        continue
                    if Lc:
                        nc.gpsimd.dma_scatter_add(
                            mebufs[it][...], st_c[:, 0:Lc // 128, :],
                            ts_sb[:, off // 16:(off + Lc) // 16], Lc, Lc, H)
                    if Lr:
                        nc.gpsimd.dma_scatter_add(
                            mebufs[it][...], st_r[:, 0:Lr // 128, :],
                            ts_sb[:, (off + Lc) // 16:(off + W) // 16], Lr, Lr, H)

            def a_phase(h_src, it):
                accs = {}
                for j in range(NA):
                    base = plan['a_bases'][j]
                    g = agp.tile([128, NCG, 128], BF16, tag='gath')
                    nc.gpsimd.dma_gather(
                        g[...], h_src[base:N, :], ag_sb[:, j * (CA // 16):(j + 1) * (CA // 16)],
                        CA, CA, H, single_packet=(CA <= 1024))
                    ohs = agp.tile([128, NCG * 128], BF16, tag='ohs')
                    nc.sync.dma_start(ohs[...], aoh[:, j * NCG * 128:(j + 1) * NCG * 128])
                    for gg in range(NCG):
                        gi = j * NCG + gg
                        dt, gstart, gstop, firstpass = plan['a_groups'][gi]
                        if gstart:
                            accs[dt] = ps.tile([128, 128], F32, name=f'acc{dt}',
                                               tag='pD' if dt % 2 == 0 else 'pE')
                        nc.tensor.matmul(accs[dt][...], ohs[:, gg * 128:(gg + 1) * 128],
                                         g[:, gg, :], start=gstart, stop=gstop)
                        if gstop:
                            if firstpass:
                                nc.vector.tensor_copy(S_sb[dt][...], accs[dt][...])
                            else:
                                nc.vector.tensor_add(S_sb[dt][...], S_sb[dt][...], accs[dt][...])

            def node_phase(h_old_src, it):
                # groups of up to 4 node tiles (free dim <= 512)
                t0 = 0
                while t0 < NTILE:
                    nt = min(4, NTILE - t0)
                    F = nt * 128
                    S_T = nodep.tile([128, 512], BF16, tag='S_T')
                    Me_T = nodep.tile([128, 512], BF16, tag='Me_T')
                    H_T = nodep.tile([128, 512], BF16, tag='H_T')
                    hots = []
                    for u in range(nt):
                        r0 = (t0 + u) * 128
                        r1 = min(r0 + 128, NS)
                        nr = r1 - r0
                        ptn = pst.tile([128, 128], BF16, tag='pT')
                        nc.tensor.transpose(ptn[...], S_sb[t0 + u][...], ident_sb[...])
                        nc.vector.tensor_copy(S_T[:, u * 128:(u + 1) * 128], ptn[...])
                        met = nodep.tile([128, 128], BF16, tag='met')
                        nc.sync.dma_start(met[:nr, :], mebufs[it][r0:r1, :])
                        nc.vector.tensor_scalar_mul(met[...], met[...], dinv_sb[:, t0 + u:t0 + u + 1])
                        ptn2 = pst.tile([128, 128], BF16, tag='pT')
                        nc.tensor.transpose(ptn2[...], met[...], ident_sb[...])
                        nc.vector.tensor_copy(Me_T[:, u * 128:(u + 1) * 128], ptn2[...])
                        hot = holdp.tile([128, 128], BF16, tag='hot')
                        nc.sync.dma_start(hot[:nr, :], h_old_src[r0:r1, :])
                        ptn3 = pst.tile([128, 128], BF16, tag='pT')
                        nc.tensor.transpose(ptn3[...], hot[...], ident_sb[...])
                        nc.vector.tensor_copy(H_T[:, u * 128:(u + 1) * 128], ptn3[...])
                        hots.append((hot, r0, r1))
                    STf = S_T[:, 0:F]; MeTf = Me_T[:, 0:F]; HTf = H_T[:, 0:F]
                    nbase = t0 * 128
                    pmn = ps.tile([128, 512], F32, tag='pA')
                    nc.tensor.matmul(pmn[:, 0:F], wnn_sb[...], STf, start=True, stop=False)
                    nc.tensor.matmul(pmn[:, 0:F], bnn_sb[...],
                                     sden_sb[0:1, nbase:nbase + F], start=False, stop=True)
                    mn_T = nodep.tile([128, 512], BF16, tag='mn_T')
                    nc.vector.tensor_copy(mn_T[:, 0:F], pmn[:, 0:F])
                    pg1 = ps.tile([128, 512], F32, tag='pB')
                    nc.tensor.matmul(pg1[:, 0:F], wg1k[0][...], MeTf, start=True, stop=False)
                    nc.tensor.matmul(pg1[:, 0:F], wg1k[1][...], mn_T[:, 0:F], start=False, stop=False)
                    nc.tensor.matmul(pg1[:, 0:F], wg1k[2][...], HTf, start=False, stop=True)
                    rg = nodep.tile([128, 512], BF16, tag='rg')
                    nc.scalar.activation(rg[:, 0:F], pg1[:, 0:F], AF.Relu, bias=bg1_sb[:, 0:1])
                    pg2 = ps.tile([1, 512], F32, tag='pC')
                    nc.tensor.matmul(pg2[:, 0:F], wg2_sb[...], rg[:, 0:F], start=True, stop=True)
                    gate = nodep.tile([1, 512], BF16, tag='gate')
                    nc.scalar.activation(gate[:, 0:F], pg2[:, 0:F], AF.Sigmoid, bias=bg2_sb[:, 0:1])
                    pgb = ps.tile([128, 512], F32, tag='pC')
                    nc.tensor.matmul(pgb[:, 0:F], ones_sb[...], gate[:, 0:F], start=True, stop=True)
                    m_T = nodep.tile([128, 512], BF16, tag='m_T')
                    nc.vector.tensor_mul(m_T[:, 0:F], pgb[:, 0:F], mn_T[:, 0:F])
                    nc.vector.tensor_add(m_T[:, 0:F], m_T[:, 0:F], MeTf)
                    # GRU
                    pr = ps.tile([128, 512], F32, tag='pD')
                    nc.tensor.matmul(pr[:, 0:F], wiht_sb[:, 0:128], m_T[:, 0:F], start=True, stop=False)
                    nc.tensor.matmul(pr[:, 0:F], whht_sb[:, 0:128], HTf, start=False, stop=True)
                    pz = ps.tile([128, 512], F32, tag='pE')
                    nc.tensor.matmul(pz[:, 0:F], wiht_sb[:, 128:256], m_T[:, 0:F], start=True, stop=False)
                    nc.tensor.matmul(pz[:, 0:F], whht_sb[:, 128:256], HTf, start=False, stop=True)
                    pgin = ps.tile([128, 512], F32, tag='pF')
                    nc.tensor.matmul(pgin[:, 0:F], wiht_sb[:, 256:384], m_T[:, 0:F], start=True, stop=True)
                    pghn = ps.tile([128, 512], F32, tag='pG')
                    nc.tensor.matmul(pghn[:, 0:F], whht_sb[:, 256:384], HTf, start=True, stop=True)
                    rr = nodep.tile([128, 512], BF16, tag='rr')
                    nc.scalar.activation(rr[:, 0:F], pr[:, 0:F], AF.Sigmoid, bias=brz_sb[:, 0:1])
                    zz = nodep.tile([128, 512], BF16, tag='zz')
                    nc.scalar.activation(zz[:, 0:F], pz[:, 0:F], AF.Sigmoid, bias=brz_sb[:, 1:2])
                    t1 = nodep.tile([128, 512], BF16, tag='t1')
                    nc.vector.tensor_scalar_add(t1[:, 0:F], pghn[:, 0:F], bhn_sb[:, 0:1])
                    nc.vector.tensor_mul(t1[:, 0:F], t1[:, 0:F], rr[:, 0:F])
                    nc.vector.tensor_add(t1[:, 0:F], t1[:, 0:F], pgin[:, 0:F])
                    nn_ = nodep.tile([128, 512], BF16, tag='nn_')
                    nc.scalar.activation(nn_[:, 0:F], t1[:, 0:F], AF.Tanh, bias=bin_sb[:, 0:1])
                    # h_new = n + z*(h - n)
                    d = nodep.tile([128, 512], BF16, tag='d')
                    nc.vector.tensor_sub(d[:, 0:F], HTf, nn_[:, 0:F])
                    nc.vector.tensor_mul(d[:, 0:F], d[:, 0:F], zz[:, 0:F])
                    nc.vector.tensor_add(d[:, 0:F], d[:, 0:F], nn_[:, 0:F])
                    for u in range(nt):
                        hot, r0, r1 = hots[u]
                        nr = r1 - r0
                        pt2 = pst.tile([128, 128], BF16, tag='pT')
                        nc.tensor.transpose(pt2[...], d[:, u * 128:(u + 1) * 128], ident_sb[...])
                        hn = nodep.tile([128, 128], BF16 if it == 0 else F32, tag='hn')
                        nc.vector.tensor_sub(hn[...], pt2[...], hot[...])
                        nc.vector.tensor_scalar_mul(hn[...], hn[...], cpl_sb[:, t0 + u:t0 + u + 1])
                        nc.vector.tensor_add(hn[...], hn[...], hot[...])
                        if it == 0:
                            nc.sync.dma_start(hsl_new[r0:r1, :], hn[:nr, :])
                        else:
                            nc.sync.dma_start(h_out[r0:r1, :], hn[:nr, :])
                    t0 += nt

            import concourse.mybir as mybir2
            skip = os.environ.get('BCGNN_SKIP', '')
            for it in range(N_ITER):
                h_src = h0 if it == 0 else h_work2
                h_old_src = hsl0 if it == 0 else hsl_new
                if TOT and 'tie' not in skip:
                    tie_phase(h_src, h_old_src, it)
                if NA and 'A' not in skip:
                    a_phase(h_src, it)
                node_phase(h_old_src, it)
                if it == 0:
                    nc.gpsimd.collective_compute(
                        'AllGather', mybir2.AluOpType.bypass,
                        replica_groups=[list(range(M))],
                        ins=[hsl_new[:, :]], outs=[h_work2[:, :]])
            if TOT:
                nc.gpsimd.dma_start(e_out[:, :], e_sb[...])

    nc.compile()
    return nc


# ----------------------------------------------------------------------------
# run + assemble
# ----------------------------------------------------------------------------

def _make_in_maps(plan, weights):
    M = plan['M']
    TOT = plan['TIE']['TOT']
    shared = dict(
        Wer1=weights['W_er1'].astype(nbf16),
        Wer2=weights['W_er2'].astype(nbf16),
        Wnn=weights['W_nn'].astype(nbf16),
        bnn=weights['b_nn'][None, :].astype(nbf16),
        Wg1=weights['W_g1'].astype(nbf16),
        Wg2=weights['W_g2'].astype(nbf16),
        WihT=weights['W_ih'].T.copy().astype(nbf16),
        WhhT=weights['W_hh'].T.copy().astype(nbf16),
        ber1=weights['b_er1'].reshape(2, 128).T.copy().astype(np.float32),
        ber2=weights['b_er2'][:, None].astype(np.float32),
        bg1=weights['b_g1'][:, None].astype(np.float32),
        bg2=weights['b_g2'].reshape(1, 1).astype(np.float32),
        brz=(weights['b_ih'] + weights['b_hh'])[:256].reshape(2, 128).T.copy().astype(np.float32),
        bin=weights['b_ih'][256:384][:, None].astype(np.float32),
        bhn=weights['b_hh'][256:384][:, None].astype(np.float32),
        ident=np.eye(128, dtype=np.float32).astype(nbf16),
        ones1=np.ones((1, 128), np.float32).astype(nbf16),
    )
    in_maps = []
    for k in range(M):
        m = dict(shared)
        m['h0'] = plan['h_bf']
        m['hsl0'] = plan['h_sl'][k]
        if plan['NA']:
            m['A_g'] = plan['A_g'][k]; m['A_oh'] = plan['A_oh'][k]
        if TOT:
            T = plan['TIE']
            m['TG_g'] = T['g'][k]; m['TG_l'] = T['l'][k]
            m['TS'] = T['s'][k]; m['TIE_e'] = T['e'][k]
        m['dinv'] = plan['dinv'][k]; m['cpl'] = plan['cpl'][k]; m['sden'] = plan['sden'][k]
        in_maps.append(m)
    return in_maps


def _assemble(plan, h, e, results):
    M = plan['M']; NS = plan['NS']
    h_out = np.concatenate([np.asarray(results[k]['h_out']) for k in range(M)], axis=0)
    h_out = h_out.astype(np.float32)
    h_out[~plan['is_cpl']] = h[~plan['is_cpl']]
    e_out = e.astype(np.float32).copy()
    if plan['TIE']['TOT']:
        for k in range(M):
            eo = np.asarray(results[k]['e_out'])      # [128, TOT] f32
            eids = plan['TIE']['eids'][k]
            vmask = eids >= 0
            e_out[eids[vmask]] = eo[:, vmask].T
    return h_out, e_out


def _run(plan, weights, use_sim=False, trace=False):
    nc = _build(plan, weights)
    in_maps = _make_in_maps(plan, weights)
    M = plan['M']
    if use_sim:
        from concourse import bass_interp
        sim = bass_interp.MultiCoreSim(nc, M)
        for k in range(M):
            for name, arr in in_maps[k].items():
                sim.cores[k].tensor(name)[:] = arr
        sim.simulate()
        results = []
        for k in range(M):
            r = {'h_out': np.array(sim.cores[k].tensor('h_out')[:])}
            if plan['TIE']['TOT']:
                r['e_out'] = np.array(sim.cores[k].tensor('e_out')[:])
            results.append(r)
        return results, None
    else:
        from concourse.bass_utils import run_bass_kernel_spmd
        out = run_bass_kernel_spmd(nc, in_maps, core_ids=list(range(M)), trace=trace)
        return out.results, out


def kernel(h, e, edge_attr_raw,
           W_er1, b_er1, W_er2, b_er2,
           W_ih, W_hh, b_ih, b_hh,
           W_nn, b_nn,
           W_g1, b_g1, W_g2, b_g2,
           edge_index, is_tie):
    h = np.asarray(h, np.float32)
    e = np.asarray(e, np.float32)
    weights = dict(W_er1=np.asarray(W_er1, np.float32), b_er1=np.asarray(b_er1, np.float32),
                   W_er2=np.asarray(W_er2, np.float32), b_er2=np.asarray(b_er2, np.float32),
                   W_ih=np.asarray(W_ih, np.float32), W_hh=np.asarray(W_hh, np.float32),
                   b_ih=np.asarray(b_ih, np.float32), b_hh=np.asarray(b_hh, np.float32),
                   W_nn=np.asarray(W_nn, np.float32), b_nn=np.asarray(b_nn, np.float32),
                   W_g1=np.asarray(W_g1, np.float32), b_g1=np.asarray(b_g1, np.float32),
                   W_g2=np.asarray(W_g2, np.float32), b_g2=np.asarray(b_g2, np.float32))
    plan = _plan(h, e, np.asarray(edge_attr_raw, np.float32),
                 np.asarray(edge_index), np.asarray(is_tie), M_CORES, CA_DEF, CT_DEF)
    results, _ = _run(plan, weights, use_sim=False)
    h_out, e_out = _assemble(plan, h, e, results)
    return h_out, e_out


# revision 27
# speedup vs baseline: 1.9034x; 1.0209x over previous
"""BCGNN (nn_BCGNN_15934328668763) Trainium2 Bass kernel, 8 NeuronCores.

Destination-sharded edge-parallel design: core k owns node slice
[k*NS, (k+1)*NS). Every segment-sum lands entirely in the owner's slice, so
no all-reduce of [N,H] partials is needed -- only one AllGather of the
updated h between the two refinement iterations.

Sparsity exploited (vs the dense reference):
  - e_work only changes on tie edges (~5% of E): refine MLP runs on those.
  - m_node_all only takes edges with w_norm != 0 (internal & is_cpl[col]).
  - all [E]/[N] scalar index prep (w_norm, deg, is_cpl, sden) is host-side
    sharding logic; all [*,H] tensor compute runs on device in bf16.
"""
import sys, os

for _p in ('/opt/trn_rl_repo', '/root/.axon_site/_ro/trn_rl_repo'):
    if os.path.isdir(_p) and _p not in sys.path:
        sys.path.insert(0, _p)

import numpy as np
import ml_dtypes

nbf16 = ml_dtypes.bfloat16

# problem constants (hardcoded; harness calls kernel() with exactly these shapes)
N_FULL = 50000
E_FULL = 800000
H = 128
M_CORES = 8
N_ITER = 2
BOUND = 32768          # int16 gather index limit (exclusive)

CA_DEF = 2048          # A-stream (m_node) chunk: indices per dma_gather
CT_DEF = 512           # tie-stream chunk


# ----------------------------------------------------------------------------
# host-side planning
# ----------------------------------------------------------------------------

def _wrap_idx(mat16):
    """[16, L] channel-wrapped int16 -> [128, L] replicated for 8 Q7 cores."""
    return np.tile(mat16, (8, 1)).astype(np.int16)


def _plan(h, e, edge_attr_raw, edge_index, is_tie, M, CA, CT):
    N, Hd = h.shape
    E = e.shape[0]
    assert Hd == H
    NS = N // M
    SA = CA // 16           # slots per channel per A-chunk
    ST = CT // 16
    row = edge_index[0].astype(np.int64)
    col = edge_index[1].astype(np.int64)
    tie = np.asarray(is_tie).astype(bool)
    tie_f = tie.astype(np.float32)

    cpl_cnt = (np.bincount(row, weights=tie_f, minlength=N)
               + np.bincount(col, weights=tie_f, minlength=N)).astype(np.float32)
    is_cpl = cpl_cnt > 0
    internal = ~tie
    to_cpl = internal & is_cpl[col]
    X = np.abs(edge_attr_raw[:, 1].astype(np.float32))
    w = np.where(to_cpl, (1.0 / np.sqrt(X * X + np.float32(1e-6))).astype(np.float32),
                 np.float32(0.0)).astype(np.float32)
    w_den = np.bincount(col, weights=w, minlength=N).astype(np.float32)
    w_norm = (w / (w_den[col] + np.float32(1e-6))).astype(np.float32)
    deg = np.maximum(cpl_cnt, np.float32(1.0))
    deg_inv = (np.float32(1.0) / deg).astype(np.float32)
    sden = np.bincount(col, weights=w_norm, minlength=N).astype(np.float32)

    core_of = lambda x: x // NS

    # ---------------- A stream (m_node): one-hot matmul accumulation ----------
    # Edges sorted by (row-bucket, dest). Each (bucket, dest-tile) run is padded
    # to a multiple of 128; a 128-position group feeds one one-hot matmul into
    # the dest-tile's PSUM accumulator. Group structure unified across cores.
    NTILE = -(-NS // 128)
    a_sel_all = to_cpl
    a_core = core_of(col)
    NCG = CA // 128
    per_core = []
    cnt = np.zeros((M, 2, NTILE), np.int64)
    for k in range(M):
        sel = a_sel_all & (a_core == k)
        r = row[sel]
        dloc = (col[sel] - k * NS).astype(np.int64)
        wv = w_norm[sel]
        bk = (r >= BOUND).astype(np.int64)
        order = np.lexsort((dloc, bk))
        r, dloc, wv, bk = r[order], dloc[order], wv[order], bk[order]
        dt = dloc // 128
        for b in (0, 1):
            cnt[k, b] = np.bincount(dt[bk == b], minlength=NTILE)
        per_core.append((r, dloc, wv, bk, dt))
    G = np.maximum(-(-cnt.max(axis=0) // 128), 0)        # [2, NTILE] groups per run
    # pad each bucket's group count to a chunk multiple (merge pad into last run)
    for b in (0, 1):
        ng = int(G[b].sum())
        if ng == 0:
            continue
        padg = (-ng) % NCG
        last = np.nonzero(G[b])[0][-1]
        G[b, last] += padg
    NA = int(G.sum()) // NCG
    a_bases = []
    for b in (0, 1):
        a_bases += [b * BOUND] * (int(G[b].sum()) // NCG)
    # group metadata: (dt, start, stop, first_pass) per global group
    a_groups = []
    gfirst = {}          # (b, dt) -> first group index
    gi = 0
    for b in (0, 1):
        for dtv in range(NTILE):
            n = int(G[b, dtv])
            if n == 0:
                continue
            gfirst[(b, dtv)] = gi
            firstpass = (b == 0) or (G[0, dtv] == 0)
            for u in range(n):
                a_groups.append((dtv, u == 0, u == n - 1, firstpass))
                gi += 1
    NGT = gi
    assert NGT == NA * NCG

    A_g, A_oh = [], []
    for k in range(M):
        g16 = np.zeros((16, NA * (CA // 16)), np.int16)
        ohmat = np.zeros((128, NGT * 128), nbf16)
        r, dloc, wv, bk, dt = per_core[k]
        run_id = bk * NTILE + dt
        # start position of each run in the unified stream
        run_base = np.zeros(2 * NTILE, np.int64)
        for b in (0, 1):
            for dtv in range(NTILE):
                if (b, dtv) in gfirst:
                    run_base[b * NTILE + dtv] = 128 * gfirst[(b, dtv)]
        # rank within run
        uniq, inv_start = np.unique(run_id, return_index=True)
        starts = np.zeros(2 * NTILE, np.int64)
        starts[uniq] = inv_start
        rank = np.arange(len(run_id)) - starts[run_id]
        pos = run_base[run_id] + rank
        g16[pos % 16, pos // 16] = (r - bk * BOUND).astype(np.int16)
        ohmat[pos % 128, (pos // 128) * 128 + (dloc - dt * 128)] = wv.astype(nbf16)
        A_g.append(_wrap_idx(g16))
        A_oh.append(ohmat)

    # ---------------- merged tie stream ----------------
    # Core k refines every tie edge whose col OR row lands in its slice (one
    # refine per edge). Each window is [both | col-only | row-only] segments
    # (each padded to 128). Scatter calls: col-dests over [both+col-only];
    # row-dests over [both] and over [row-only]. Greedy window assignment
    # keeps col-dests and row-dests duplicate-free inside every window.
    tidx_all = np.nonzero(tie)[0]
    CSEG = 1024          # per-window capacity: col-owned entries
    RSEG = 1024          # row-owned entries

    def build_tie_merged():
        # Entries: (core, seg): seg 0 = col-owned (dest=col, global endpoint=row),
        # seg 1 = row-owned (dest=row, global endpoint=col). Edges owned through
        # both endpoints appear once in each segment (refine is recomputed; the
        # two e copies evolve identically). The local endpoint (always inside
        # the owner slice) is gathered from the core's own h slice, so only the
        # global endpoint needs the int16 bucket split -> 2 window groups.
        per_core = []
        for k in range(M):
            ents = []        # per entry: (gsrc, lsrc, dest, eid, seg)
            for seg, own, gcol in ((0, col, row), (1, row, col)):
                sel = (own[tidx_all] // NS) == k
                te = tidx_all[sel]
                ents.append((gcol[te], own[te] - k * NS, own[te] - k * NS, te, seg))
            per_core.append(ents)
        # greedy window packing per bucket group of the global endpoint
        nwin = [0, 0]
        slots = [[None, None] for _ in range(M)]
        demand = {}
        for k in range(M):
            for seg in (0, 1):
                gsrc, lsrc, dest, te, _ = per_core[k][seg]
                bk = (gsrc >= BOUND).astype(np.int64)
                cap = CSEG if seg == 0 else RSEG
                out = np.zeros((len(te), 2), np.int64)
                state = {0: ([], []), 1: ([], [])}     # bucket -> (fills, used-sets)
                for i in range(len(te)):
                    fills, useds = state[bk[i]]
                    w = 0
                    while True:
                        if w == len(fills):
                            fills.append(0)
                            useds.append(set())
                        if fills[w] < cap and dest[i] not in useds[w]:
                            out[i] = (bk[i] * 1000 + w, fills[w])
                            fills[w] += 1
                            useds[w].add(dest[i])
                            break
                        w += 1
                for b in (0, 1):
                    fills, _ = state[b]
                    nwin[b] = max(nwin[b], len(fills))
                    for w, f in enumerate(fills):
                        key = (b, w, seg)
                        demand[key] = max(demand.get(key, 0), f)
                slots[k][seg] = out
        r128 = lambda x: -(-x // 128) * 128
        win_meta = []        # (bucket, off, Lc, Lr)
        off = 0
        offmap = {}
        for b in (0, 1):
            for w in range(nwin[b]):
                Lc = r128(demand.get((b, w, 0), 0))
                Lr = r128(demand.get((b, w, 1), 0))
                if Lc + Lr == 0:
                    continue
                win_meta.append((b, off, Lc, Lr))
                offmap[(b, w)] = (off, Lc, Lr)
                off += Lc + Lr
        TOT = off
        Gg, Gl, Ts, Edat, Eids = [], [], [], [], []
        for k in range(M):
            g16 = np.zeros((16, TOT // 16), np.int16)
            l16 = np.zeros((16, TOT // 16), np.int16)
            s16 = np.full((16, TOT // 16), NS, np.int16)
            ed = np.zeros((128, TOT), nbf16)
            eid = np.full((TOT,), -1, np.int64)
            for seg in (0, 1):
                gsrc, lsrc, dest, te, _ = per_core[k][seg]
                sl = slots[k][seg]
                if len(te) == 0:
                    continue
                b = sl[:, 0] // 1000
                w = sl[:, 0] % 1000
                base = np.array([offmap[(bb, ww)][0] + (0 if seg == 0 else offmap[(bb, ww)][1])
                                 for bb, ww in zip(b, w)], np.int64)
                pos = base + sl[:, 1]
                g16[pos % 16, pos // 16] = (gsrc - b * BOUND).astype(np.int16)
                l16[pos % 16, pos // 16] = lsrc.astype(np.int16)
                s16[pos % 16, pos // 16] = dest.astype(np.int16)
                ed[:, pos] = e[te].T.astype(nbf16)
                if seg == 0:
                    eid[pos] = te
            Gg.append(_wrap_idx(g16))
            Gl.append(_wrap_idx(l16))
            Ts.append(_wrap_idx(s16))
            Edat.append(ed)
            Eids.append(eid)
        return dict(TOT=TOT, win=win_meta, g=Gg, l=Gl, s=Ts, e=Edat, eids=Eids)

    TIE = build_tie_merged()

    # ---------------- node-slice arrays ----------------
    NSP = NTILE * 128
    dinv_nm, cpl_nm, sden_fm, h_sl = [], [], [], []
    for k in range(M):
        sl = slice(k * NS, (k + 1) * NS)
        di = np.ones(NSP, np.float32); di[:NS] = deg_inv[sl]
        cm = np.zeros(NSP, np.float32); cm[:NS] = is_cpl[sl].astype(np.float32)
        sd = np.zeros(NSP, np.float32); sd[:NS] = sden[sl]
        dinv_nm.append(di.reshape(NTILE, 128).T.copy())     # [128, NTILE]
        cpl_nm.append(cm.reshape(NTILE, 128).T.copy())
        sden_fm.append(sd[None, :].astype(nbf16))            # [1, NSP]
        h_sl.append(h[sl].astype(nbf16))

    plan = dict(
        N=N, E=E, M=M, NS=NS, CA=CA, CT=CT, NA=NA, a_bases=a_bases,
        a_groups=a_groups, NTILE=NTILE, NSP=NSP, NGT=NGT,
        A_g=A_g, A_oh=A_oh, TIE=TIE,
        dinv=dinv_nm, cpl=cpl_nm, sden=sden_fm, h_sl=h_sl,
        is_cpl=is_cpl,
        h_bf=h.astype(nbf16),
    )
    return plan


# ----------------------------------------------------------------------------
# device graph
# ----------------------------------------------------------------------------

def _build(plan, weights):
    import concourse.bass as bass
    import concourse.bacc as bacc
    import concourse.mybir as mybir
    from concourse.tile import TileContext
    from concourse import library_config

    BF16 = mybir.dt.bfloat16
    F32 = mybir.dt.float32
    I16 = mybir.dt.int16
    AF = mybir.ActivationFunctionType

    N = plan['N']; NS = plan['NS']; M = plan['M']
    CA = plan['CA']; CT = plan['CT']; NA = plan['NA']
    NTILE = plan['NTILE']
    SROWS = NS + 16
    TOT = plan['TIE']['TOT']
    bg2_val = float(weights['b_g2'][0])

    nc = bacc.Bacc(None, target_bir_lowering=False, num_devices=M,
                   dynamic_dma_scratch_size=32768)

    P = {}
    def inp(name, shape, dt):
        P[name] = nc.declare_dram_parameter(name, list(shape), dt, isOutput=False)
        return P[name]

    h0 = inp('h0', [N, H], BF16)
    hsl0 = inp('hsl0', [NS, H], BF16)
    NGT = plan['NGT']
    ag = inp('A_g', [128, NA * (CA // 16)], I16) if NA else None
    aoh = inp('A_oh', [128, NGT * 128], BF16) if NA else None
    if TOT:
        inp('TG_g', [128, TOT // 16], I16)
        inp('TG_l', [128, TOT // 16], I16)
        inp('TS', [128, TOT // 16], I16)
        inp('TIE_e', [128, TOT], BF16)
    dinv = inp('dinv', [128, NTILE], F32)
    cplm = inp('cpl', [128, NTILE], F32)
    sdenp = inp('sden', [1, plan['NSP']], BF16)
    wer1 = inp('Wer1', [384, 256], BF16)
    wer2 = inp('Wer2', [256, 128], BF16)
    wnn = inp('Wnn', [128, 128], BF16)
    bnnp = inp('bnn', [1, 128], BF16)
    wg1 = inp('Wg1', [384, 128], BF16)
    wg2 = inp('Wg2', [128, 1], BF16)
    wiht = inp('WihT', [128, 384], BF16)
    whht = inp('WhhT', [128, 384], BF16)
    ber1 = inp('ber1', [128, 2], F32)
    ber2 = inp('ber2', [128, 1], F32)
    bg1 = inp('bg1', [128, 1], F32)
    bg2p = inp('bg2', [1, 1], F32)
    brz = inp('brz', [128, 2], F32)
    bin_ = inp('bin', [128, 1], F32)
    bhn = inp('bhn', [128, 1], F32)
    identp = inp('ident', [128, 128], BF16)
    onesp = inp('ones1', [1, 128], BF16)
    # zero-initialized tie-scatter accumulators (device zeroes internal DRAM)
    mebufs = [nc.dram_tensor(f'me{it}', [SROWS, H], BF16) for it in range(N_ITER)]

    h_out = nc.declare_dram_parameter('h_out', [NS, H], F32, isOutput=True)
    e_out = (nc.declare_dram_parameter('e_out', [128, TOT], F32, isOutput=True)
             if TOT else None)

    hsl_new = nc.dram_tensor('hsl_new', [NS, H], BF16)
    h_work2 = nc.dram_tensor('h_work2', [N, H], BF16, addr_space='Shared')

    with TileContext(nc) as tc:
        nc.gpsimd.load_library(library_config.mlp)
        with tc.tile_pool(name='res', bufs=1) as res, \
             tc.tile_pool(name='agp', bufs=3) as agp, \
             tc.tile_pool(name='tie', bufs=3) as tiep, \
             tc.tile_pool(name='node', bufs=2) as nodep, \
             tc.tile_pool(name='hold', bufs=10) as holdp, \
             tc.tile_pool(name='ps', bufs=1, space='PSUM') as ps:
            pst = ps

            def load_res(pname, shape, dt, tag):
                t = res.tile(list(shape), dt, tag=tag)
                nc.sync.dma_start(t[...], P[pname][...])
                return t

            # zero the tie-scatter accumulators
            ZC = 1024
            zt = res.tile([128, ZC], BF16, tag='zt')
            nc.vector.memset(zt[...], 0.0)
            for tgt in mebufs:
                for r0 in range(0, SROWS, ZC):
                    nr = min(ZC, SROWS - r0)
                    nc.sync.dma_start(tgt[r0:r0 + nr, :], zt[:, 0:nr])

            # resident loads
            ag_sb = load_res('A_g', [128, NA * (CA // 16)], I16, 'ag') if NA else None
            S_sb = [res.tile([128, 128], BF16, tag=f'Ssb{t}', name=f'Ssb{t}') for t in range(NTILE)]
            if TOT:
                tg_g = load_res('TG_g', [128, TOT // 16], I16, 'tgg')
                tg_l = load_res('TG_l', [128, TOT // 16], I16, 'tgl')
                ts_sb = load_res('TS', [128, TOT // 16], I16, 'tss')
                e_sb = load_res('TIE_e', [128, TOT], BF16, 'tiee')
            dinv_sb = load_res('dinv', [128, NTILE], F32, 'dinv')
            cpl_sb = load_res('cpl', [128, NTILE], F32, 'cpl')
            sden_sb = load_res('sden', [1, plan['NSP']], BF16, 'sden')
            ident_sb = load_res('ident', [128, 128], BF16, 'ident')
            ones_sb = load_res('ones1', [1, 128], BF16, 'ones')
            wnn_sb = load_res('Wnn', [128, 128], BF16, 'wnn')
            bnn_sb = load_res('bnn', [1, 128], BF16, 'bnn')
            wg2_sb = load_res('Wg2', [128, 1], BF16, 'wg2')
            wiht_sb = load_res('WihT', [128, 384], BF16, 'wiht')
            whht_sb = load_res('WhhT', [128, 384], BF16, 'whht')
            ber1_sb = load_res('ber1', [128, 2], F32, 'ber1')
            ber2_sb = load_res('ber2', [128, 1], F32, 'ber2')
            bg1_sb = load_res('bg1', [128, 1], F32, 'bg1')
            bg2_sb = load_res('bg2', [1, 1], F32, 'bg2')
            brz_sb = load_res('brz', [128, 2], F32, 'brz')
            bin_sb = load_res('bin', [128, 1], F32, 'bin')
            bhn_sb = load_res('bhn', [128, 1], F32, 'bhn')

            w1k = []
            for kk in range(3):
                t = res.tile([128, 256], BF16, tag=f'w1k{kk}')
                nc.sync.dma_start(t[...], P['Wer1'][kk * 128:(kk + 1) * 128, :])
                w1k.append(t)
            w2k = []
            for kk in range(2):
                t = res.tile([128, 128], BF16, tag=f'w2k{kk}')
                nc.sync.dma_start(t[...], P['Wer2'][kk * 128:(kk + 1) * 128, :])
                w2k.append(t)
            wg1k = []
            for kk in range(3):
                t = res.tile([128, 128], BF16, tag=f'wg1k{kk}')
                nc.sync.dma_start(t[...], P['Wg1'][kk * 128:(kk + 1) * 128, :])
                wg1k.append(t)

            NCG = CA // 128   # col-groups per A chunk
            NTG = CT // 128

            def tie_phase(h_src, h_old_src, it):
                for (bb, off, Lc, Lr) in plan['TIE']['win']:
                    W = Lc + Lr
                    g1 = tiep.tile([128, 1, W], BF16, tag='g1', name=f'g1_{it}_{off}')
                    nc.gpsimd.dma_gather(
                        g1[...], h_src[bb * BOUND:N, :], tg_g[:, off // 16:(off + W) // 16],
                        W, W, H, transpose=True, single_packet=(W <= 1024))
                    g2 = tiep.tile([128, 1, W], BF16, tag='g2', name=f'g2_{it}_{off}')
                    nc.gpsimd.dma_gather(
                        g2[...], h_old_src[:, :], tg_l[:, off // 16:(off + W) // 16],
                        W, W, H, transpose=True, single_packet=(W <= 1024))
                    st_c = tiep.tile([128, max(Lc, 128) // 128, 128], BF16, tag='stc',
                                     name=f'stc_{it}_{off}')
                    st_r = tiep.tile([128, max(Lr, 128) // 128, 128], BF16, tag='str',
                                     name=f'str_{it}_{off}')
                    for seg, s0g, Ls in ((0, 0, Lc), (1, Lc, Lr)):
                        for s0 in range(0, Ls, CT):
                            sw = min(CT, Ls - s0)
                            a0 = s0g + s0
                            ecol = e_sb[:, off + a0:off + a0 + sw]
                            hrow = (g1 if seg == 0 else g2)[:, 0, a0:a0 + sw]
                            hcol = (g2 if seg == 0 else g1)[:, 0, a0:a0 + sw]
                            p1a = pst.tile([128, CT], F32, tag='pA')
                            p1b = pst.tile([128, CT], F32, tag='pB')
                            for pp, wsl in ((p1a, slice(0, 128)), (p1b, slice(128, 256))):
                                nc.tensor.matmul(pp[:, 0:sw], w1k[0][:, wsl], ecol, start=True, stop=False)
                                nc.tensor.matmul(pp[:, 0:sw], w1k[1][:, wsl], hrow, start=False, stop=False)
                                nc.tensor.matmul(pp[:, 0:sw], w1k[2][:, wsl], hcol, start=False, stop=True)
                            r1a = tiep.tile([128, CT], BF16, tag='r1a')
                            nc.scalar.activation(r1a[:, 0:sw], p1a[:, 0:sw], AF.Relu, bias=ber1_sb[:, 0:1])
                            r1b = tiep.tile([128, CT], BF16, tag='r1b')
                            nc.scalar.activation(r1b[:, 0:sw], p1b[:, 0:sw], AF.Relu, bias=ber1_sb[:, 1:2])
                            p2 = pst.tile([128, CT], F32, tag='pC')
                            nc.tensor.matmul(p2[:, 0:sw], w2k[0][...], r1a[:, 0:sw], start=True, stop=False)
                            nc.tensor.matmul(p2[:, 0:sw], w2k[1][...], r1b[:, 0:sw], start=False, stop=True)
                            tmp = tiep.tile([128, CT], BF16, tag='etmp')
                            nc.vector.tensor_scalar_add(tmp[:, 0:sw], p2[:, 0:sw], ber2_sb[:, 0:1])
                            nc.vector.tensor_add(ecol, ecol, tmp[:, 0:sw])
                            stt = st_c if seg == 0 else st_r
                            for bb2 in range(s0 // 128, (s0 + sw) // 128):
                                pt = pst.tile([128, 128], BF16, tag='pT')
                                nc.tensor.transpose(pt[...], e_sb[:, off + s0g + bb2 * 128:off + s0g + (bb2 + 1) * 128], ident_sb[...])
                                nc.vector.tensor_copy(stt[:, bb2, :], pt[...])
                    if 'tsc' in os.environ.get('BCGNN_SKIP', ''):
                        continue
                    if Lc:
                        nc.gpsimd.dma_scatter_add(
                            mebufs[it][...], st_c[:, 0:Lc // 128, :],
                            ts_sb[:, off // 16:(off + Lc) // 16], Lc, Lc, H)
                    if Lr:
                        nc.gpsimd.dma_scatter_add(
                            mebufs[it][...], st_r[:, 0:Lr // 128, :],
                            ts_sb[:, (off + Lc) // 16:(off + W) // 16], Lr, Lr, H)

            def a_phase(h_src, it):
                accs = {}
                for j in range(NA):
                    base = plan['a_bases'][j]
                    g = agp.tile([128, NCG, 128], BF16, tag='gath')
                    nc.gpsimd.dma_gather(
                        g[...], h_src[base:N, :], ag_sb[:, j * (CA // 16):(j + 1) * (CA // 16)],
                        CA, CA, H, single_packet=(CA <= 1024))
                    ohs = agp.tile([128, NCG * 128], BF16, tag='ohs')
                    nc.sync.dma_start(ohs[...], aoh[:, j * NCG * 128:(j + 1) * NCG * 128])
                    for gg in range(NCG):
                        gi = j * NCG + gg
                        dt, gstart, gstop, firstpass = plan['a_groups'][gi]
                        if gstart:
                            accs[dt] = ps.tile([128, 128], F32, name=f'acc{dt}',
                                               tag='pD' if dt % 2 == 0 else 'pE')
                        nc.tensor.matmul(accs[dt][...], ohs[:, gg * 128:(gg + 1) * 128],
                                         g[:, gg, :], start=gstart, stop=gstop)
                        if gstop:
                            if firstpass:
                                nc.vector.tensor_copy(S_sb[dt][...], accs[dt][...])
                            else:
                                nc.vector.tensor_add(S_sb[dt][...], S_sb[dt][...], accs[dt][...])

            def node_phase(h_old_src, it):
                # groups of up to 4 node tiles (free dim <= 512)
                t0 = 0
                while t0 < NTILE:
                    nt = min(4, NTILE - t0)
                    F = nt * 128
                    S_T = nodep.tile([128, 512], BF16, tag='S_T')
                    Me_T = nodep.tile([128, 512], BF16, tag='Me_T')
                    H_T = nodep.tile([128, 512], BF16, tag='H_T')
                    hots = []
                    for u in range(nt):
                        r0 = (t0 + u) * 128
                        r1 = min(r0 + 128, NS)
                        nr = r1 - r0
                        ptn = pst.tile([128, 128], BF16, tag='pT')
                        nc.tensor.transpose(ptn[...], S_sb[t0 + u][...], ident_sb[...])
                        nc.vector.tensor_copy(S_T[:, u * 128:(u + 1) * 128], ptn[...])
                        met = nodep.tile([128, 128], BF16, tag='met')
                        nc.sync.dma_start(met[:nr, :], mebufs[it][r0:r1, :])
                        nc.vector.tensor_scalar_mul(met[...], met[...], dinv_sb[:, t0 + u:t0 + u + 1])
                        ptn2 = pst.tile([128, 128], BF16, tag='pT')
                        nc.tensor.transpose(ptn2[...], met[...], ident_sb[...])
                        nc.vector.tensor_copy(Me_T[:, u * 128:(u + 1) * 128], ptn2[...])
                        hot = holdp.tile([128, 128], BF16, tag='hot')
                        nc.sync.dma_start(hot[:nr, :], h_old_src[r0:r1, :])
                        ptn3 = pst.tile([128, 128], BF16, tag='pT')
                        nc.tensor.transpose(ptn3[...], hot[...], ident_sb[...])
                        nc.vector.tensor_copy(H_T[:, u * 128:(u + 1) * 128], ptn3[...])
                        hots.append((hot, r0, r1))
                    STf = S_T[:, 0:F]; MeTf = Me_T[:, 0:F]; HTf = H_T[:, 0:F]
                    nbase = t0 * 128
                    pmn = ps.tile([128, 512], F32, tag='pA')
                    nc.tensor.matmul(pmn[:, 0:F], wnn_sb[...], STf, start=True, stop=False)
                    nc.tensor.matmul(pmn[:, 0:F], bnn_sb[...],
                                     sden_sb[0:1, nbase:nbase + F], start=False, stop=True)
                    mn_T = nodep.tile([128, 512], BF16, tag='mn_T')
                    nc.vector.tensor_copy(mn_T[:, 0:F], pmn[:, 0:F])
                    pg1 = ps.tile([128, 512], F32, tag='pB')
                    nc.tensor.matmul(pg1[:, 0:F], wg1k[0][...], MeTf, start=True, stop=False)
                    nc.tensor.matmul(pg1[:, 0:F], wg1k[1][...], mn_T[:, 0:F], start=False, stop=False)
                    nc.tensor.matmul(pg1[:, 0:F], wg1k[2][...], HTf, start=False, stop=True)
                    rg = nodep.tile([128, 512], BF16, tag='rg')
                    nc.scalar.activation(rg[:, 0:F], pg1[:, 0:F], AF.Relu, bias=bg1_sb[:, 0:1])
                    pg2 = ps.tile([1, 512], F32, tag='pC')
                    nc.tensor.matmul(pg2[:, 0:F], wg2_sb[...], rg[:, 0:F], start=True, stop=True)
                    gate = nodep.tile([1, 512], BF16, tag='gate')
                    nc.scalar.activation(gate[:, 0:F], pg2[:, 0:F], AF.Sigmoid, bias=bg2_sb[:, 0:1])
                    pgb = ps.tile([128, 512], F32, tag='pC')
                    nc.tensor.matmul(pgb[:, 0:F], ones_sb[...], gate[:, 0:F], start=True, stop=True)
                    m_T = nodep.tile([128, 512], BF16, tag='m_T')
                    nc.vector.tensor_mul(m_T[:, 0:F], pgb[:, 0:F], mn_T[:, 0:F])
                    nc.vector.tensor_add(m_T[:, 0:F], m_T[:, 0:F], MeTf)
                    # GRU
                    pr = ps.tile([128, 512], F32, tag='pD')
                    nc.tensor.matmul(pr[:, 0:F], wiht_sb[:, 0:128], m_T[:, 0:F], start=True, stop=False)
                    nc.tensor.matmul(pr[:, 0:F], whht_sb[:, 0:128], HTf, start=False, stop=True)
                    pz = ps.tile([128, 512], F32, tag='pE')
                    nc.tensor.matmul(pz[:, 0:F], wiht_sb[:, 128:256], m_T[:, 0:F], start=True, stop=False)
                    nc.tensor.matmul(pz[:, 0:F], whht_sb[:, 128:256], HTf, start=False, stop=True)
                    pgin = ps.tile([128, 512], F32, tag='pF')
                    nc.tensor.matmul(pgin[:, 0:F], wiht_sb[:, 256:384], m_T[:, 0:F], start=True, stop=True)
                    pghn = ps.tile([128, 512], F32, tag='pG')
                    nc.tensor.matmul(pghn[:, 0:F], whht_sb[:, 256:384], HTf, start=True, stop=True)
                    rr = nodep.tile([128, 512], BF16, tag='rr')
                    nc.scalar.activation(rr[:, 0:F], pr[:, 0:F], AF.Sigmoid, bias=brz_sb[:, 0:1])
                    zz = nodep.tile([128, 512], BF16, tag='zz')
                    nc.scalar.activation(zz[:, 0:F], pz[:, 0:F], AF.Sigmoid, bias=brz_sb[:, 1:2])
                    t1 = nodep.tile([128, 512], BF16, tag='t1')
                    nc.vector.tensor_scalar_add(t1[:, 0:F], pghn[:, 0:F], bhn_sb[:, 0:1])
                    nc.vector.tensor_mul(t1[:, 0:F], t1[:, 0:F], rr[:, 0:F])
                    nc.vector.tensor_add(t1[:, 0:F], t1[:, 0:F], pgin[:, 0:F])
                    nn_ = nodep.tile([128, 512], BF16, tag='nn_')
                    nc.scalar.activation(nn_[:, 0:F], t1[:, 0:F], AF.Tanh, bias=bin_sb[:, 0:1])
                    # h_new = n + z*(h - n)
                    d = nodep.tile([128, 512], BF16, tag='d')
                    nc.vector.tensor_sub(d[:, 0:F], HTf, nn_[:, 0:F])
                    nc.vector.tensor_mul(d[:, 0:F], d[:, 0:F], zz[:, 0:F])
                    nc.vector.tensor_add(d[:, 0:F], d[:, 0:F], nn_[:, 0:F])
                    for u in range(nt):
                        hot, r0, r1 = hots[u]
                        nr = r1 - r0
                        pt2 = pst.tile([128, 128], BF16, tag='pT')
                        nc.tensor.transpose(pt2[...], d[:, u * 128:(u + 1) * 128], ident_sb[...])
                        hn = nodep.tile([128, 128], BF16 if it == 0 else F32, tag='hn')
                        nc.vector.tensor_sub(hn[...], pt2[...], hot[...])
                        nc.vector.tensor_scalar_mul(hn[...], hn[...], cpl_sb[:, t0 + u:t0 + u + 1])
                        nc.vector.tensor_add(hn[...], hn[...], hot[...])
                        if it == 0:
                            nc.sync.dma_start(hsl_new[r0:r1, :], hn[:nr, :])
                        else:
                            nc.sync.dma_start(h_out[r0:r1, :], hn[:nr, :])
                    t0 += nt

            import concourse.mybir as mybir2
            skip = os.environ.get('BCGNN_SKIP', '')
            for it in range(N_ITER):
                h_src = h0 if it == 0 else h_work2
                h_old_src = hsl0 if it == 0 else hsl_new
                if TOT and 'tie' not in skip:
                    tie_phase(h_src, h_old_src, it)
                if NA and 'A' not in skip:
                    a_phase(h_src, it)
                node_phase(h_old_src, it)
                if it == 0:
                    nc.gpsimd.collective_compute(
                        'AllGather', mybir2.AluOpType.bypass,
                        replica_groups=[list(range(M))],
                        ins=[hsl_new[:, :]], outs=[h_work2[:, :]])
            if TOT:
                nc.gpsimd.dma_start(e_out[:, :], e_sb[...])

    nc.compile()
    return nc


# ----------------------------------------------------------------------------
# run + assemble
# ----------------------------------------------------------------------------

def _make_in_maps(plan, weights):
    M = plan['M']
    TOT = plan['TIE']['TOT']
    shared = dict(
        Wer1=weights['W_er1'].astype(nbf16),
        Wer2=weights['W_er2'].astype(nbf16),
        Wnn=weights['W_nn'].astype(nbf16),
        bnn=weights['b_nn'][None, :].astype(nbf16),
        Wg1=weights['W_g1'].astype(nbf16),
        Wg2=weights['W_g2'].astype(nbf16),
        WihT=weights['W_ih'].T.copy().astype(nbf16),
        WhhT=weights['W_hh'].T.copy().astype(nbf16),
        ber1=weights['b_er1'].reshape(2, 128).T.copy().astype(np.float32),
        ber2=weights['b_er2'][:, None].astype(np.float32),
        bg1=weights['b_g1'][:, None].astype(np.float32),
        bg2=weights['b_g2'].reshape(1, 1).astype(np.float32),
        brz=(weights['b_ih'] + weights['b_hh'])[:256].reshape(2, 128).T.copy().astype(np.float32),
        bin=weights['b_ih'][256:384][:, None].astype(np.float32),
        bhn=weights['b_hh'][256:384][:, None].astype(np.float32),
        ident=np.eye(128, dtype=np.float32).astype(nbf16),
        ones1=np.ones((1, 128), np.float32).astype(nbf16),
    )
    in_maps = []
    for k in range(M):
        m = dict(shared)
        m['h0'] = plan['h_bf']
        m['hsl0'] = plan['h_sl'][k]
        if plan['NA']:
            m['A_g'] = plan['A_g'][k]; m['A_oh'] = plan['A_oh'][k]
        if TOT:
            T = plan['TIE']
            m['TG_g'] = T['g'][k]; m['TG_l'] = T['l'][k]
            m['TS'] = T['s'][k]; m['TIE_e'] = T['e'][k]
        m['dinv'] = plan['dinv'][k]; m['cpl'] = plan['cpl'][k]; m['sden'] = plan['sden'][k]
        in_maps.append(m)
    return in_maps


def _assemble(plan, h, e, results):
    M = plan['M']; NS = plan['NS']
    h_out = np.concatenate([np.asarray(results[k]['h_out']) for k in range(M)], axis=0)
    h_out = h_out.astype(np.float32)
    h_out[~plan['is_cpl']] = h[~plan['is_cpl']]
    e_out = e.astype(np.float32).copy()
    if plan['TIE']['TOT']:
        for k in range(M):
            eo = np.asarray(results[k]['e_out'])      # [128, TOT] f32
            eids = plan['TIE']['eids'][k]
            vmask = eids >= 0
            e_out[eids[vmask]] = eo[:, vmask].T
    return h_out, e_out


def _run(plan, weights, use_sim=False, trace=False):
    nc = _build(plan, weights)
    in_maps = _make_in_maps(plan, weights)
    M = plan['M']
    if use_sim:
        from concourse import bass_interp
        sim = bass_interp.MultiCoreSim(nc, M)
        for k in range(M):
            for name, arr in in_maps[k].items():
                sim.cores[k].tensor(name)[:] = arr
        sim.simulate()
        results = []
        for k in range(M):
            r = {'h_out': np.array(sim.cores[k].tensor('h_out')[:])}
            if plan['TIE']['TOT']:
                r['e_out'] = np.array(sim.cores[k].tensor('e_out')[:])
            results.append(r)
        return results, None
    else:
        from concourse.bass_utils import run_bass_kernel_spmd
        out = run_bass_kernel_spmd(nc, in_maps, core_ids=list(range(M)), trace=trace)
        return out.results, out


def kernel(h, e, edge_attr_raw,
           W_er1, b_er1, W_er2, b_er2,
           W_ih, W_hh, b_ih, b_hh,
           W_nn, b_nn,
           W_g1, b_g1, W_g2, b_g2,
           edge_index, is_tie):
    h = np.asarray(h, np.float32)
    e = np.asarray(e, np.float32)
    weights = dict(W_er1=np.asarray(W_er1, np.float32), b_er1=np.asarray(b_er1, np.float32),
                   W_er2=np.asarray(W_er2, np.float32), b_er2=np.asarray(b_er2, np.float32),
                   W_ih=np.asarray(W_ih, np.float32), W_hh=np.asarray(W_hh, np.float32),
                   b_ih=np.asarray(b_ih, np.float32), b_hh=np.asarray(b_hh, np.float32),
                   W_nn=np.asarray(W_nn, np.float32), b_nn=np.asarray(b_nn, np.float32),
                   W_g1=np.asarray(W_g1, np.float32), b_g1=np.asarray(b_g1, np.float32),
                   W_g2=np.asarray(W_g2, np.float32), b_g2=np.asarray(b_g2, np.float32))
    plan = _plan(h, e, np.asarray(edge_attr_raw, np.float32),
                 np.asarray(edge_index), np.asarray(is_tie), M_CORES, CA_DEF, CT_DEF)
    results, _ = _run(plan, weights, use_sim=False)
    h_out, e_out = _assemble(plan, h, e, results)
    return h_out, e_out
